# revision 1
# baseline (speedup 1.0000x reference)
"""GSA video block kernel for 8 TRN2 NeuronCores.

Sharding: head-parallel attention (2 heads/core) -> one AllToAll that
redistributes the RMS-normed head outputs from head-sharded to
token-sharded -> token-parallel tail (out-proj + LN2 + MLP with full
weights, 256 tokens/core).

The sequential T=512 gated-slot-attention scan is replaced by an exact
chunk-parallel formulation (C=128): intra-chunk terms via causal-masked
matmuls with per-slot decay factors, inter-chunk via carried states
K[DK,M] / V[M,DV].
"""

import os
import sys

import numpy as np
import ml_dtypes

if "/opt/trn_rl_repo" not in sys.path:
    sys.path.insert(0, "/opt/trn_rl_repo")

import concourse.bass as bass  # noqa: E402
import concourse.mybir as mybir  # noqa: E402
import concourse.tile as tile  # noqa: E402
from concourse import bacc  # noqa: E402
from concourse.bass_utils import run_bass_kernel_spmd  # noqa: E402

BF16 = mybir.dt.bfloat16
F32 = mybir.dt.float32
AF = mybir.ActivationFunctionType
ALU = mybir.AluOpType
AX = mybir.AxisListType

B, T, D = 4, 512, 1024
H, DK, DV, M = 16, 64, 64, 64
MLP = 4096
EPS = 1e-6

N_CORES = 8
C = 128                    # scan chunk length
NCH = T // C               # chunks per batch = 4
TOK = B * T                # 2048 flat tokens
TT = TOK // 128            # 16 token tiles
DT = D // 128              # 8 d tiles
MT = MLP // 128            # 32 mlp tiles
TAIL = TOK // N_CORES      # 256 tokens per core in the tail
LN8 = float(np.log(0.125))
RG = [list(range(N_CORES))]

_cache = {}


def _emit(nc, tc, io):
    x_bf, x_res = io["x_bf"], io["x_res"]
    wq, wk, wv, wf = io["wq"], io["wk"], io["wv"], io["wf"]
    bqp, bkp = io["bqp"], io["bkp"]
    bvp, bfp, b1row = io["bvp"], io["bfp"], io["b1row"]
    wo, w1, b1v, w2, b2v = io["wo"], io["w1"], io["b1v"], io["w2"], io["b2v"]
    ltriT, onescol, cmask = io["ltriT"], io["onescol"], io["cmask"]
    ident, bd128, ones_row = io["ident"], io["bd128"], io["ones_row"]
    y_out, dump = io["y_out"], io["dump"]
    P = 128

    const = tc.alloc_tile_pool(name="const", bufs=1)
    persist = tc.alloc_tile_pool(name="persist", bufs=1)
    dram = tc.alloc_tile_pool(name="dram", bufs=1, space="DRAM")

    # ---- warmup collective (prepay ncfw handshake) -----------------------
    wa_in = dram.tile([8, 128], BF16, name="wa_in")
    wa_out = dram.tile([8, 128], BF16, name="wa_out")
    nc.gpsimd.collective_compute("AllReduce", ALU.add, replica_groups=RG,
                                 ins=[wa_in.opt()], outs=[wa_out.opt()])

    # ---- constants into SBUF --------------------------------------------
    def cload(ap, shape, dt, name):
        t = const.tile(shape, dt, name=name)
        nc.sync.dma_start(t[:], ap)
        return t

    ltriT_sb = cload(ltriT.ap(), [128, 128], F32, "ltriT")
    onescol_sb = cload(onescol.ap(), [128, 1], F32, "onescol")
    cmask_sb = cload(cmask.ap(), [128, 128], BF16, "cmask")
    ident_sb = cload(ident.ap(), [128, 128], BF16, "ident")
    bd128_sb = cload(bd128.ap(), [128, 128], BF16, "bd128")
    ones_row_sb = cload(ones_row.ap(), [1, 128], BF16, "ones_row")
    bqp_sb = cload(bqp.ap(), [128, 1], F32, "bqp")
    bkp_sb = cload(bkp.ap(), [128, 1], F32, "bkp")
    bvp_sb = cload(bvp.ap(), [128, 1], F32, "bvp")
    bfp_sb = cload(bfp.ap(), [128, 1], F32, "bfp")
    b1row_sb = cload(b1row.ap(), [1, MLP], BF16, "b1row")
    b1_sb = cload(b1v.ap(), [128, MT], F32, "b1")
    eps_sb = const.tile([128, 1], F32)
    nc.vector.memset(eps_sb[:], EPS)
    ln8_sb = const.tile([128, 1], F32)
    nc.vector.memset(ln8_sb[:], LN8)

    wq_sb = const.tile([128, DT, 128], BF16)
    nc.sync.dma_start(wq_sb[:], wq.ap().rearrange("(dt p) j -> p dt j", p=P))
    wk_sb = const.tile([128, DT, 128], BF16)
    nc.sync.dma_start(wk_sb[:], wk.ap().rearrange("(dt p) j -> p dt j", p=P))
    wv_sb = const.tile([128, DT, 128], BF16)
    nc.sync.dma_start(wv_sb[:], wv.ap().rearrange("(dt p) j -> p dt j", p=P))
    wf_sb = const.tile([128, DT, 128], BF16)
    nc.sync.dma_start(wf_sb[:], wf.ap().rearrange("(dt p) j -> p dt j", p=P))
    wo_sb = const.tile([128, DT, D], BF16)
    nc.sync.dma_start(wo_sb[:], wo.ap().rearrange("(dt p) j -> p dt j", p=P))

    # ---- persistent activation tensors ----------------------------------
    qT = persist.tile([128, TOK], BF16, name="qT")       # [2h*64 dk, t]
    kT = persist.tile([128, TOK], BF16, name="kT")
    k_tm = persist.tile([128, TT, 128], BF16, name="k_tm")   # [t, 2h*64]
    v_tm = persist.tile([128, TT, 128], BF16, name="v_tm")
    vT = persist.tile([128, TOK], BF16, name="vT")
    fT = persist.tile([128, TOK], BF16, name="fT")
    f_tm = persist.tile([128, TT, 128], BF16, name="f_tm")
    sp = persist.tile([128, TT, 128], F32, name="sp")        # softplus(-f)
    s_tm = persist.tile([128, TT, 128], BF16, name="s_tm")   # 1-exp(g)
    onT = persist.tile([128, TOK], BF16, name="onT")         # normed oT

    h_dram = [dram.tile([TOK // 2, D], BF16, name=f"h_dram{g}")
              for g in range(2)]
    k_dram = dram.tile([128, TOK], BF16, name="k_dram")
    v_dram = dram.tile([128, TOK], BF16, name="v_dram")
    f_dram = dram.tile([128, TOK], BF16, name="f_dram")
    z_dram = dram.tile([TAIL, MLP], BF16, name="z_dram")
    a2a_in = dram.tile([128 * N_CORES, TAIL], BF16, name="a2a_in")
    a2a_out = dram.tile([128 * N_CORES, TAIL], BF16, name="a2a_out")
    h2d = dram.tile([TAIL, D], BF16, name="h2d")

    # =====================================================================
    # P1: LN1 stats (batched sqrt) + normalize, store h to DRAM
    # =====================================================================
    p0 = tc.alloc_tile_pool(name="p0", bufs=1)
    x_sb = p0.tile([128, TT, D], BF16, name="x_sb")
    stats = p0.tile([128, 2, TT], F32, name="stats")  # [.,0,:]=r [.,1,:]=nrmu
    with tc.tile_pool(name="p1", bufs=3) as p1, \
         tc.tile_pool(name="p1s", bufs=2) as p1s:
        musb = p1.tile([128, TT], F32, name="musb")
        sssb = p1.tile([128, TT], F32, name="sssb")
        for tt in range(TT):
            nc.sync.dma_start(
                x_sb[:, tt, :],
                x_bf.ap().rearrange("(n p) d -> n p d", p=P)[tt])
            nc.vector.tensor_reduce(musb[:, tt:tt + 1], x_sb[:, tt, :],
                                    AX.X, ALU.add)
            sq = p1s.tile([128, D], BF16, name="sq")
            nc.scalar.activation(sq[:], x_sb[:, tt, :], AF.Square,
                                 accum_out=sssb[:, tt:tt + 1])
        mu = p1.tile([128, TT], F32, name="mu")
        nc.vector.tensor_scalar_mul(mu[:], musb[:], 1.0 / D)
        var = p1.tile([128, TT], F32, name="var")
        nc.vector.tensor_tensor(var[:], mu[:], mu[:], ALU.mult)
        ex2 = p1.tile([128, TT], F32, name="ex2")
        nc.vector.tensor_scalar_mul(ex2[:], sssb[:], 1.0 / D)
        nc.vector.tensor_tensor(var[:], ex2[:], var[:], ALU.subtract)
        sd = p1.tile([128, TT], F32, name="sd")
        nc.scalar.activation(sd[:], var[:], AF.Sqrt, bias=eps_sb[:])
        nc.vector.reciprocal(stats[:, 0, :], sd[:])
        nc.vector.tensor_tensor(stats[:, 1, :], stats[:, 0, :], mu[:],
                                ALU.mult)
        nc.vector.tensor_scalar_mul(stats[:, 1, :], stats[:, 1, :], -1.0)
        for tt in range(TT):
            g, i = tt // (TT // 2), tt % (TT // 2)
            ht = p1.tile([128, D], BF16, name="ht")
            nc.scalar.activation(ht[:], x_sb[:, tt, :], AF.Identity,
                                 bias=stats[:, 1, tt:tt + 1],
                                 scale=stats[:, 0, tt:tt + 1])
            nc.sync.dma_start(
                h_dram[g][:].rearrange("(n p) d -> n p d", p=P)[i], ht[:])

    p0.release()
    # =====================================================================
    # P2: transpose-load hT, projections
    # =====================================================================
    p2h = tc.alloc_tile_pool(name="p2h", bufs=1)
    hT = p2h.tile([128, DT, TOK], BF16, name="hT")
    for g in range(2):
        for dt in range(DT):
            nc.sync.dma_start_transpose(
                hT[:, dt, g * (TOK // 2):(g + 1) * (TOK // 2)],
                h_dram[g][:, dt * 128:(dt + 1) * 128])

    with tc.tile_pool(name="proj_ps", bufs=2, space="PSUM") as pps:
        for tci in range(4):
            tcsl = slice(tci * 512, (tci + 1) * 512)
            for (dst, w_sb, bias, fn) in (
                    (qT, wq_sb, bqp_sb, AF.Silu),
                    (kT, wk_sb, bkp_sb, AF.Silu),
                    (vT, wv_sb, bvp_sb, AF.Identity),
                    (fT, wf_sb, bfp_sb, AF.Identity)):
                bank = pps.tile([128, 512], F32, name="projbank")
                for dt in range(DT):
                    nc.tensor.matmul(bank[:], w_sb[:, dt, :], hT[:, dt, tcsl],
                                     start=(dt == 0), stop=(dt == DT - 1))
                nc.scalar.activation(dst[:, tcsl], bank[:], fn,
                                     bias=bias[:], scale=1.0)

    p2h.release()
    # roundtrip k/v/f to token-major via DMA transpose
    nc.sync.dma_start(k_dram[:], kT[:])
    nc.sync.dma_start(v_dram[:], vT[:])
    nc.sync.dma_start(f_dram[:], fT[:])
    for tt in range(TT):
        ttsl = slice(tt * 128, (tt + 1) * 128)
        nc.sync.dma_start_transpose(k_tm[:, tt, :], k_dram[:, ttsl])
        nc.sync.dma_start_transpose(v_tm[:, tt, :], v_dram[:, ttsl])
        nc.sync.dma_start_transpose(f_tm[:, tt, :], f_dram[:, ttsl])

    with tc.tile_pool(name="sgate", bufs=2) as sg:
        for tci in range(4):
            csl = slice(tci * 4, (tci + 1) * 4)
            enf = sg.tile([128, 4, 128], F32, name="enf")
            nc.scalar.activation(enf[:], f_tm[:, csl, :], AF.Exp,
                                 scale=-1.0)
            nc.scalar.activation(sp[:, csl, :], enf[:], AF.Ln, bias=1.0)
            e8 = sg.tile([128, 4, 128], BF16, name="e8")
            nc.scalar.activation(e8[:], sp[:, csl, :], AF.Exp, scale=-0.125)
            nc.vector.tensor_scalar(s_tm[:, csl, :],
                                    e8[:], -1.0, 1.0, ALU.mult, ALU.add)

    for nm, t_sb in (("qT", qT), ("kT", kT)):
        if (d := dump(nm, [128, TOK], BF16)) is not None:
            nc.sync.dma_start(d.ap(), t_sb[:])
    for nm, t_sb in (("k_tm", k_tm), ("v_tm", v_tm), ("s_tm", s_tm)):
        if (d := dump(nm, [128, TT * 128], BF16)) is not None:
            nc.sync.dma_start(d.ap().rearrange("p (n f) -> p n f", n=TT),
                              t_sb[:])
    if (d := dump("sp", [128, TT * 128])) is not None:
        nc.sync.dma_start(d.ap().rearrange("p (n f) -> p n f", n=TT), sp[:])

    # =====================================================================
    # P3: chunked scan, b-major; RMS + write o_nT
    # =====================================================================
    with tc.tile_pool(name="scan_ps", bufs=2, space="PSUM") as sps, \
         tc.tile_pool(name="scan_sb", bufs=2) as ssb, \
         tc.tile_pool(name="state_sb", bufs=1) as stb:
        Kst = stb.tile([128, 64], BF16, name="Kst")   # [2h*64 dk, s]
        Vst = stb.tile([128, 64], BF16, name="Vst")   # [2h*64 s, dv]
        for b in range(B):
            for c in range(NCH):
                bi = b * 4 + c
                tsl = slice(b * 512 + c * 128, b * 512 + (c + 1) * 128)
                first = (c == 0)
                bankA = sps.tile([128, 512], F32, name="bankA", bufs=1)
                ps_b = bankA[:, 0:128]
                ps_ss = bankA[:, 128:256]
                ps_lc = bankA[:, 256:257]
                ps_lcr = bankA[0:1, 257:385]
                ps_lambc = bankA[:, 384:512]
                bankB = sps.tile([128, 512], F32, name="bankB")
                ps_a = (bankB[:, 0:128], bankB[:, 128:256])
                ps_ok = bankB[:, 256:384]
                bankD = sps.tile([128, 1024], BF16, name="bankD", bufs=1)
                ps_pt = (bankD[0:64, 0:128], bankD[0:64, 128:256])
                ps_st = (bankD[0:64, 256:384], bankD[0:64, 384:512])
                bankE = sps.tile([128, 512], F32, name="bankE", bufs=1)
                ps_b2 = (bankE[:, 0:128], bankE[:, 128:256])
                bankF = sps.tile([128, 512], F32, name="bankF")
                ps_o = (bankF[0:64, 0:128], bankF[0:64, 128:256])
                ps_dk = (bankF[0:64, 256:320], bankF[0:64, 320:384])
                ps_dv = (bankF[0:64, 384:448], bankF[0:64, 448:512])

                # cumsum b = ltriT.T @ sp (f32); colsum -> bCT
                nc.tensor.matmul(ps_b, ltriT_sb[:], sp[:, bi, :],
                                 start=True, stop=True)
                nc.tensor.matmul(ps_lc, sp[:, bi, :], onescol_sb[:],
                                 start=True, stop=True)
                lam = ssb.tile([128, 128], BF16, name="lam")
                nc.scalar.activation(lam[:], ps_b, AF.Exp)
                lam_s = ssb.tile([128, 128], BF16, name="lam_s")
                nc.vector.tensor_scalar_mul(lam_s[:], lam[:], 0.125)
                en = ssb.tile([128, 128], BF16, name="en")
                with nc.allow_low_precision(reason="en=1/lam feeds bf16"):
                    nc.vector.reciprocal(en[:], lam[:])
                lamCT = ssb.tile([128, 1], F32, name="lamCT")
                nc.scalar.activation(lamCT[:], ps_lc, AF.Exp)
                nc.tensor.matmul(ps_lcr, onescol_sb[:], sp[:, bi, :],
                                 start=True, stop=True)
                lamCr = ssb.tile([1, 128], BF16, name="lamCr")
                nc.scalar.activation(lamCr[:], ps_lcr, AF.Exp)

                s_til = ssb.tile([128, 128], BF16, name="s_til")
                nc.vector.tensor_tensor(s_til[:], s_tm[:, bi, :], en[:],
                                        ALU.mult)
                nc.tensor.matmul(ps_lambc, ones_row_sb[:], lamCr[:],
                                 start=True, stop=True)
                s2 = ssb.tile([128, 128], BF16, name="s2")
                nc.vector.tensor_tensor(s2[:], s_til[:], ps_lambc, ALU.mult)

                am = ssb.tile([128, 256], BF16, name="am")
                for h in range(2):
                    hs = slice(h * 64, (h + 1) * 64)
                    nc.tensor.matmul(ps_a[h], kT[hs, tsl], qT[hs, tsl],
                                     start=True, stop=True)
                    nc.vector.tensor_tensor(am[:, h * 128:(h + 1) * 128],
                                            ps_a[h], cmask_sb[:], ALU.mult)
                for h in range(2):
                    hs = slice(h * 64, (h + 1) * 64)
                    oks = ps_ok[:, h * 64:(h + 1) * 64]
                    if not first:
                        nc.tensor.matmul(oks, qT[hs, tsl], Kst[hs, :],
                                         start=True, stop=False)
                    nc.tensor.matmul(oks, am[:, h * 128:(h + 1) * 128],
                                     s_til[:, h * 64:(h + 1) * 64],
                                     start=first, stop=True)
                # softmax over slots (per head), pl = P * lam
                oksc = ssb.tile([128, 128], F32, name="oksc")
                nc.vector.tensor_tensor(oksc[:], ps_ok, lam_s[:], ALU.mult)
                ex = ssb.tile([128, 128], BF16, name="ex")
                nc.scalar.activation(ex[:], oksc[:], AF.Exp)
                rsum = ssb.tile([128, 2], F32, name="rsum")
                nc.vector.tensor_reduce(
                    rsum[:], ex[:].rearrange("p (h s) -> p h s", h=2),
                    AX.X, ALU.add)
                rcp = ssb.tile([128, 2], F32, name="rcp")
                nc.vector.reciprocal(rcp[:], rsum[:])
                pl = ssb.tile([128, 128], BF16, name="pl")
                nc.vector.tensor_tensor(pl[:], ex[:], lam[:], ALU.mult)
                nc.vector.tensor_tensor(
                    pl[:].rearrange("p (h s) -> p h s", h=2),
                    pl[:].rearrange("p (h s) -> p h s", h=2),
                    rcp[:].rearrange("p (h o) -> p h o", h=2)
                        .to_broadcast([128, 2, 64]),
                    ALU.mult)

                # transposes: plT, s_tilT  [2h*64 s, 128 t]
                plT = ssb.tile([128, 128], BF16, name="plT")
                s_tilT = ssb.tile([128, 128], BF16, name="s_tilT")
                for h in range(2):
                    hs = slice(h * 64, (h + 1) * 64)
                    nc.tensor.transpose(ps_pt[h], pl[:, hs], ident_sb[:])
                    nc.vector.tensor_copy(plT[hs, :], ps_pt[h])
                    nc.tensor.transpose(ps_st[h], s_til[:, hs], ident_sb[:])
                    nc.vector.tensor_copy(s_tilT[hs, :], ps_st[h])

                b2m = ssb.tile([128, 256], BF16, name="b2m")
                for h in range(2):
                    hs = slice(h * 64, (h + 1) * 64)
                    nc.tensor.matmul(ps_b2[h], s_tilT[hs, :], plT[hs, :],
                                     start=True, stop=True)
                    nc.vector.tensor_tensor(b2m[:, h * 128:(h + 1) * 128],
                                            ps_b2[h], cmask_sb[:], ALU.mult)
                for h in range(2):
                    hs = slice(h * 64, (h + 1) * 64)
                    if not first:
                        nc.tensor.matmul(ps_o[h], Vst[hs, :], plT[hs, :],
                                         start=True, stop=False)
                    nc.tensor.matmul(ps_o[h], v_tm[:, bi, hs],
                                     b2m[:, h * 128:(h + 1) * 128],
                                     start=first, stop=True)
                    nc.tensor.matmul(ps_dk[h], k_tm[:, bi, hs],
                                     s2[:, hs], start=True, stop=True)
                    nc.tensor.matmul(ps_dv[h], s2[:, hs], v_tm[:, bi, hs],
                                     start=True, stop=True)
                    if first:
                        nc.vector.tensor_copy(Kst[hs, :], ps_dk[h])
                        nc.vector.tensor_copy(Vst[hs, :], ps_dv[h])
                    else:
                        nc.vector.tensor_tensor(
                            Kst[hs, :], Kst[hs, :],
                            ps_lambc[hs, hs], ALU.mult)
                        nc.vector.tensor_tensor(Kst[hs, :], Kst[hs, :],
                                                ps_dk[h], ALU.add)
                        nc.vector.tensor_scalar(Vst[hs, :], Vst[hs, :],
                                                lamCT[hs, 0:1], None, ALU.mult)
                        nc.vector.tensor_tensor(Vst[hs, :], Vst[hs, :],
                                                ps_dv[h], ALU.add)

                # write raw oT (RMS batched after the loop)
                nc.vector.tensor_copy(onT[0:64, tsl], ps_o[0])
                nc.vector.tensor_copy(onT[64:128, tsl], ps_o[1])

    # batched RMS over dv for the whole oT
    with tc.tile_pool(name="rms_ps", bufs=2, space="PSUM") as rps, \
         tc.tile_pool(name="rms_sb", bufs=2) as rsb:
        for q4 in range(4):
            qsl = slice(q4 * 512, (q4 + 1) * 512)
            sqo = rsb.tile([128, 512], BF16, name="sqo")
            nc.vector.tensor_tensor(sqo[:], onT[:, qsl], onT[:, qsl],
                                    ALU.mult)
            ps_ss = rps.tile([128, 512], F32, name="ps_ss")
            nc.tensor.matmul(ps_ss[:], bd128_sb[:], sqo[:],
                             start=True, stop=True)
            sdo = rsb.tile([128, 512], F32, name="sdo")
            nc.scalar.activation(sdo[:], ps_ss[:], AF.Sqrt,
                                 bias=eps_sb[:], scale=1.0 / DV)
            rro = rsb.tile([128, 512], F32, name="rro")
            nc.vector.reciprocal(rro[:], sdo[:])
            nc.vector.tensor_tensor(onT[:, qsl], onT[:, qsl], rro[:],
                                    ALU.mult)

    if (d := dump("onT", [128, TOK], BF16)) is not None:
        nc.sync.dma_start(d.ap(), onT[:])

    # head-sharded -> token-sharded redistribution
    nc.sync.dma_start(
        a2a_in[:].rearrange("(r p) t -> p r t", p=P),
        onT[:].rearrange("p (r t) -> p r t", r=N_CORES))
    nc.gpsimd.collective_compute("AllToAll", ALU.bypass, replica_groups=RG,
                                 ins=[a2a_in.opt()], outs=[a2a_out.opt()])

    # =====================================================================
    # P4 tail: out-proj + residual + LN2 + MLP on 256 tokens
    # =====================================================================
    with tc.tile_pool(name="tail_ps", bufs=1, space="PSUM") as tps, \
         tc.tile_pool(name="tail_sb", bufs=2) as tsb, \
         tc.tile_pool(name="tail_keep", bufs=1) as tkb, \
         tc.tile_pool(name="w1stream", bufs=3) as w1s, \
         tc.tile_pool(name="w2stream", bufs=4) as w2s:
        ofT = tkb.tile([128, DT, TAIL], BF16, name="ofT")
        nc.sync.dma_start(ofT[:],
                          a2a_out[:].rearrange("(jt p) t -> p jt t", p=P))
        x2 = tkb.tile([128, 2, D], F32, name="x2")
        nc.sync.dma_start(x2[:],
                          x_res.ap().rearrange("(n p) d -> p n d", p=P))

        op_bank = tps.tile([128, 512], F32, name="op_bank")
        for tt2 in range(2):
            for nb in range(2):
                nsl = slice(nb * 512, (nb + 1) * 512)
                for jt in range(DT):
                    nc.tensor.matmul(op_bank[:],
                                     ofT[:, jt, tt2 * 128:(tt2 + 1) * 128],
                                     wo_sb[:, jt, nsl],
                                     start=(jt == 0), stop=(jt == DT - 1))
                nc.vector.tensor_tensor(x2[:, tt2, nsl], op_bank[:],
                                        x2[:, tt2, nsl], ALU.add)
        if (d := dump("x2", [128, 2 * D])) is not None:
            nc.sync.dma_start(d.ap().rearrange("p (n f) -> p n f", n=2),
                              x2[:])

        # LN2 + store h2, transpose-load
        h2T = tkb.tile([128, DT, TAIL], BF16, name="h2T")
        for tt2 in range(2):
            x2t = x2[:, tt2, :]
            ssum = tsb.tile([128, 1], F32, name="ssum2")
            nc.vector.tensor_reduce(ssum[:], x2t, AX.X, ALU.add)
            sq = tsb.tile([128, D], BF16, name="sq2")
            ssq = tsb.tile([128, 1], F32, name="ssq2")
            nc.scalar.activation(sq[:], x2t, AF.Square, accum_out=ssq[:])
            mu = tsb.tile([128, 1], F32, name="mu2")
            nc.vector.tensor_scalar_mul(mu[:], ssum[:], 1.0 / D)
            var = tsb.tile([128, 1], F32, name="var2")
            nc.vector.tensor_tensor(var[:], mu[:], mu[:], ALU.mult)
            ex2 = tsb.tile([128, 1], F32, name="ex22")
            nc.vector.tensor_scalar_mul(ex2[:], ssq[:], 1.0 / D)
            nc.vector.tensor_tensor(var[:], ex2[:], var[:], ALU.subtract)
            sd = tsb.tile([128, 1], F32, name="sd2")
            nc.scalar.activation(sd[:], var[:], AF.Sqrt, bias=eps_sb[:])
            r2 = tsb.tile([128, 1], F32, name="r2")
            nc.vector.reciprocal(r2[:], sd[:])
            nrmu = tsb.tile([128, 1], F32, name="nrmu2")
            nc.vector.tensor_tensor(nrmu[:], r2[:], mu[:], ALU.mult)
            nc.vector.tensor_scalar_mul(nrmu[:], nrmu[:], -1.0)
            h2t = tsb.tile([128, D], BF16, name="h2t")
            nc.scalar.activation(h2t[:], x2t, AF.Identity,
                                 bias=nrmu[:], scale=r2[:])
            nc.sync.dma_start(
                h2d[:].rearrange("(n p) d -> n p d", p=P)[tt2], h2t[:])
        for dt in range(DT):
            nc.sync.dma_start_transpose(h2T[:, dt, :],
                                        h2d[:, dt * 128:(dt + 1) * 128])

        # MLP1: y1 token-major [t, mlp-chunk], gelu, roundtrip to zT
        z_tm = tkb.tile([128, 2, MLP], BF16, name="z_tm")
        for mc in range(8):
            mcsl = slice(mc * 512, (mc + 1) * 512)
            w1t = w1s.tile([128, DT, 512], BF16, name="w1t")
            nc.sync.dma_start(
                w1t[:], w1.ap().rearrange("(dt p) m -> p dt m", p=P)
                [:, :, mcsl])
            for tt2 in range(2):
                y1b = tps.tile([128, 512], F32, name="y1b", bufs=2)
                for dt in range(DT):
                    nc.tensor.matmul(y1b[:],
                                     h2T[:, dt, tt2 * 128:(tt2 + 1) * 128],
                                     w1t[:, dt, :],
                                     start=(dt == 0), stop=False)
                nc.tensor.matmul(y1b[:], ones_row_sb[:], b1row_sb[:, mcsl],
                                 start=False, stop=True)
                nc.scalar.activation(z_tm[:, tt2, mcsl], y1b[:], AF.Gelu)
        nc.sync.dma_start(
            z_dram[:].rearrange("(n p) m -> p n m", p=P), z_tm[:])
        zT = tkb.tile([128, MT, TAIL], BF16, name="zT")
        for mt in range(MT):
            nc.sync.dma_start_transpose(
                zT[:, mt, :], z_dram[:, mt * 128:(mt + 1) * 128])

        # MLP2: y2 = z @ w2, accumulate over mt into 4 resident banks
        y2_banks = [tps.tile([128, 512], F32, name=f"y2b{i}")
                    for i in range(4)]
        for mt in range(MT):
            w2t = w2s.tile([128, D], BF16, name="w2t")
            nc.sync.dma_start(
                w2t[:], w2.ap().rearrange("(n p) d -> n p d", p=P)[mt])
            for tt2 in range(2):
                for nb in range(2):
                    nc.tensor.matmul(
                        y2_banks[tt2 * 2 + nb],
                        zT[:, mt, tt2 * 128:(tt2 + 1) * 128],
                        w2t[:, nb * 512:(nb + 1) * 512],
                        start=(mt == 0), stop=(mt == MT - 1))
        for tt2 in range(2):
            for nb in range(2):
                nsl = slice(nb * 512, (nb + 1) * 512)
                ys = tsb.tile([128, 512], F32, name="ys")
                nc.vector.tensor_tensor(ys[:], y2_banks[tt2 * 2 + nb],
                                        x2[:, tt2, nsl], ALU.add)
                nc.sync.dma_start(
                    y_out.ap().rearrange("(n p) d -> p n d", p=P)
                    [:, tt2, nsl], ys[:])

    for pool in (dram, persist, const):
        pool.release()


def _build():
    nc = bacc.Bacc("TRN2", target_bir_lowering=False, debug=False,
                   num_devices=N_CORES)

    def din(name, shape, dt=BF16):
        return nc.dram_tensor(name, shape, dt, kind="ExternalInput")

    io = dict(
        x_bf=din("x_bf", [TOK, D]),
        x_res=din("x_res", [TAIL, D], F32),
        wq=din("wq", [D, 128]), wk=din("wk", [D, 128]),
        wv=din("wv", [D, 128]), wf=din("wf", [D, 128]),
        bqp=din("bqp", [128, 1], F32), bkp=din("bkp", [128, 1], F32),
        bvp=din("bvp", [128, 1], F32), bfp=din("bfp", [128, 1], F32),
        b1row=din("b1row", [1, MLP]),
        wo=din("wo", [D, D]),
        w1=din("w1", [D, MLP]),
        b1v=din("b1v", [128, MLP // 128], F32),
        w2=din("w2", [MLP, D]),
        b2v=din("b2v", [1, D], F32),
        ltriT=din("ltriT", [128, 128], F32),
        onescol=din("onescol", [128, 1], F32),
        cmask=din("cmask", [128, 128]),
        ident=din("ident", [128, 128]),
        bd128=din("bd128", [128, 128]),
        ones_row=din("ones_row", [1, 128]),
        y_out=nc.dram_tensor("y_out", [TAIL, D], F32, kind="ExternalOutput"),
    )

    dbg = [s for s in os.environ.get("GSA_DEBUG", "").split(",") if s]
    dbg_outs = {}

    def dump(name, shape, dt=F32):
        if name in dbg:
            t = nc.dram_tensor("dbg_" + name, shape, dt,
                               kind="ExternalOutput")
            dbg_outs[name] = t
            return t
        return None

    io["dump"] = dump
    with tile.TileContext(nc) as tcx:
        _emit(nc, tcx, io)
    nc.compile()
    return nc, sorted(dbg_outs)


def _host_prep(inputs):
    """Fold norms/biases into weights; build per-core in_maps."""
    f32 = np.float32
    bf16 = ml_dtypes.bfloat16
    x = np.asarray(inputs["hidden_states"], f32).reshape(TOK, D)
    ln1_w = np.asarray(inputs["ln1_w"], f32)
    ln1_b = np.asarray(inputs["ln1_b"], f32)
    ln2_w = np.asarray(inputs["ln2_w"], f32)
    ln2_b = np.asarray(inputs["ln2_b"], f32)
    gnorm = np.asarray(inputs["gnorm_w"], f32)
    Wq = np.asarray(inputs["Wq"], f32) * ln1_w[:, None]
    Wk = np.asarray(inputs["Wk"], f32) * ln1_w[:, None]
    Wv = np.asarray(inputs["Wv"], f32) * ln1_w[:, None]
    Wf = np.asarray(inputs["Wf"], f32) * ln1_w[:, None]
    bq = ln1_b @ np.asarray(inputs["Wq"], f32)
    bk = ln1_b @ np.asarray(inputs["Wk"], f32)
    bv = ln1_b @ np.asarray(inputs["Wv"], f32)
    bf_ = ln1_b @ np.asarray(inputs["Wf"], f32)
    Wo = np.asarray(inputs["Wo"], f32) * np.tile(gnorm, H)[:, None]
    W1 = np.asarray(inputs["W1"], f32) * ln2_w[:, None]
    b1 = np.asarray(inputs["b1"], f32) + ln2_b @ np.asarray(inputs["W1"], f32)
    W2 = np.asarray(inputs["W2"], f32)
    b2 = np.asarray(inputs["b2"], f32)

    tri = np.tril(np.ones((128, 128), f32))  # [t, tau] tau<=t
    ltriT = np.ascontiguousarray((-0.125 * tri).T)           # [tau, t]
    cmask = np.ascontiguousarray(tri.T.astype(bf16))         # [tau, t]
    ident = np.eye(128, dtype=bf16)
    bd128 = np.kron(np.eye(2, dtype=f32),
                    np.ones((64, 64), f32)).astype(bf16)
    common = dict(
        x_bf=np.ascontiguousarray(x.astype(bf16)),
        ltriT=ltriT,
        onescol=np.full((128, 1), -0.125, f32),
        cmask=cmask, ident=ident, bd128=bd128,
        ones_row=np.ones((1, 128), bf16),
        wo=np.ascontiguousarray(Wo.astype(bf16)),
        w1=np.ascontiguousarray(W1.astype(bf16)),
        b1v=np.ascontiguousarray(b1.reshape(MLP // 128, 128).T.astype(f32)),
        b1row=np.ascontiguousarray(b1.reshape(1, MLP).astype(bf16)),
        w2=np.ascontiguousarray(W2.astype(bf16)),
        b2v=np.ascontiguousarray(b2.reshape(1, D)),
    )
    in_maps = []
    for r in range(N_CORES):
        jsl = slice(r * 128, (r + 1) * 128)  # 2 heads = 128 cols
        m = dict(common)
        m["x_res"] = np.ascontiguousarray(x[r * TAIL:(r + 1) * TAIL]
                                          + b2[None, :])
        m["wq"] = np.ascontiguousarray(Wq[:, jsl].astype(bf16))
        m["wk"] = np.ascontiguousarray(Wk[:, jsl].astype(bf16))
        m["wv"] = np.ascontiguousarray(Wv[:, jsl].astype(bf16))
        m["wf"] = np.ascontiguousarray(Wf[:, jsl].astype(bf16))
        m["bqp"] = np.ascontiguousarray(bq[jsl].reshape(128, 1))
        m["bkp"] = np.ascontiguousarray(bk[jsl].reshape(128, 1))
        m["bvp"] = np.ascontiguousarray(bv[jsl].reshape(128, 1))
        m["bfp"] = np.ascontiguousarray(bf_[jsl].reshape(128, 1))
        in_maps.append(m)
    return in_maps


def kernel(**inputs):
    if "nc" not in _cache:
        _cache["nc"], _cache["dbg"] = _build()
    nc = _cache["nc"]
    in_maps = _host_prep(inputs)
    res = run_bass_kernel_spmd(nc, in_maps, core_ids=list(range(N_CORES)),
                               trace=bool(os.environ.get("GSA_TRACE")))
    _cache["last_results"] = res
    out = np.concatenate([res.results[r]["y_out"] for r in range(N_CORES)],
                         axis=0)
    return out.reshape(B, T, D)



# revision 41
# speedup vs baseline: 1.2611x; 1.2611x over previous
"""GSA video block kernel for 8 TRN2 NeuronCores — batch-pair sharding.

Cores pair up: group g = {2g, 2g+1} owns batch g end-to-end. Within a
group each core computes 8 heads (4 head-blocks of 2) of the gated-slot
attention over the batch's 512 tokens; a pair-local AllToAll then
redistributes head outputs to token halves for the fused out-proj + LN2
+ MLP tail (256 tokens per core, full MLP weights streamed from HBM).

The T=512 scan runs chunk-parallel (C=128) exactly as the reference:
intra-chunk causal-masked matmuls with per-slot decay, inter-chunk via
carried states K[DK,M] / V[M,DV] per head.

All transposes run on the PE array (identity matmul) — no DMA
transposes. Positive-value reciprocals use exp(-ln(x)) on the scalar
engine instead of DVE reciprocal.
"""

import os
import sys

import numpy as np
import ml_dtypes

if "/opt/trn_rl_repo" not in sys.path:
    sys.path.insert(0, "/opt/trn_rl_repo")

import concourse.bass as bass  # noqa: E402
import concourse.mybir as mybir  # noqa: E402
import concourse.tile as tile  # noqa: E402
from concourse import bacc  # noqa: E402
from concourse.bass_utils import run_bass_kernel_spmd  # noqa: E402

BF16 = mybir.dt.bfloat16
F32 = mybir.dt.float32
AF = mybir.ActivationFunctionType
ALU = mybir.AluOpType
AX = mybir.AxisListType

B, T, D = 4, 512, 1024
H, DK, DV, M = 16, 64, 64, 64
MLP = 4096
EPS = 1e-6

N_CORES = 8
C = 128                    # scan chunk length
NCH = T // C               # chunks per batch = 4
HB = 4                     # head-blocks per core (2 heads each)
TAIL = 256                 # tokens per core in the tail
DT = D // 128              # 8 d tiles
RG = [list(range(N_CORES))]

_cache = {}


def _emit(nc, tc, io):
    x_t, x_res = io["x_t"], io["x_res"]
    wq, wk, wv, wf = io["wq"], io["wk"], io["wv"], io["wf"]
    bqp, bkp, bvp, bfp = io["bqp"], io["bkp"], io["bvp"], io["bfp"]
    wo, w1, b1row, w2 = io["wo"], io["w1"], io["b1row"], io["w2"]
    ltriT, onescol, onescol1 = io["ltriT"], io["onescol"], io["onescol1"]
    cmask, ident, bd128, ones_row = (io["cmask"], io["ident"], io["bd128"],
                                     io["ones_row"])
    y_out, dump = io["y_out"], io["dump"]
    P = 128

    const = tc.alloc_tile_pool(name="const", bufs=1)
    persist = tc.alloc_tile_pool(name="persist", bufs=1)
    dram = tc.alloc_tile_pool(name="dram", bufs=1, space="DRAM")

    # ---- warmup collective (prepay ncfw handshake) -----------------------
    wa_in = dram.tile([8, 128], BF16, name="wa_in")
    wa_out = dram.tile([8, 128], BF16, name="wa_out")
    nc.gpsimd.collective_compute("AllToAll", ALU.bypass, replica_groups=RG,
                                 ins=[wa_in.opt()], outs=[wa_out.opt()])

    # ---- constants into SBUF --------------------------------------------
    def cload(ap, shape, dt, name):
        t = const.tile(shape, dt, name=name)
        nc.sync.dma_start(t[:], ap)
        return t

    ltriT_sb = cload(ltriT.ap(), [128, 128], F32, "ltriT")
    onescol_sb = cload(onescol.ap(), [128, 1], F32, "onescol")
    ones1_sb = cload(onescol1.ap(), [128, 1], BF16, "ones1")
    cmask_sb = cload(cmask.ap(), [128, 128], BF16, "cmask")
    ident_sb = cload(ident.ap(), [128, 128], BF16, "ident")
    bd128_sb = cload(bd128.ap(), [128, 128], BF16, "bd128")
    ones_row_sb = cload(ones_row.ap(), [1, 128], BF16, "ones_row")
    bqp_sb = cload(bqp.ap(), [128, HB], F32, "bqp")
    bkp_sb = cload(bkp.ap(), [128, HB], F32, "bkp")
    bvp_sb = cload(bvp.ap(), [128, HB], F32, "bvp")
    bfp_sb = cload(bfp.ap(), [128, HB], F32, "bfp")
    b1row_sb = cload(b1row.ap(), [1, MLP], BF16, "b1row")
    eps_sb = const.tile([128, 1], F32)
    nc.vector.memset(eps_sb[:], EPS)

    wq_sb = const.tile([128, DT, HB, 128], BF16)
    nc.sync.dma_start(wq_sb[:], wq.ap().rearrange(
        "p (dt hb j) -> p dt hb j", dt=DT, hb=HB))
    wk_sb = const.tile([128, DT, HB, 128], BF16)
    nc.sync.dma_start(wk_sb[:], wk.ap().rearrange(
        "p (dt hb j) -> p dt hb j", dt=DT, hb=HB))
    wv_sb = const.tile([128, DT, HB, 128], BF16)
    nc.sync.dma_start(wv_sb[:], wv.ap().rearrange(
        "p (dt hb j) -> p dt hb j", dt=DT, hb=HB))
    wf_sb = const.tile([128, DT, HB, 128], BF16)
    nc.sync.dma_start(wf_sb[:], wf.ap().rearrange(
        "p (dt hb j) -> p dt hb j", dt=DT, hb=HB))
    wo_sb = const.tile([128, DT, D], BF16)
    nc.sync.dma_start(wo_sb[:], wo.ap().rearrange("p (jt n) -> p jt n", jt=DT))

    # ---- persistent activation tensors ----------------------------------
    qT = persist.tile([128, HB, T], BF16, name="qT")     # [2h*64 dk, hb, t]
    kT = persist.tile([128, HB, T], BF16, name="kT")
    k_tm = persist.tile([128, HB, NCH, 128], BF16, name="k_tm")  # [t,hb,c,j]
    v_tm = persist.tile([128, HB, NCH, 128], BF16, name="v_tm")
    sp = persist.tile([128, HB, NCH, 128], F32, name="sp")       # softplus(-f)
    s_tm = persist.tile([128, HB, NCH, 128], BF16, name="s_tm")  # 1-exp(g)
    onT = persist.tile([128, HB, T], BF16, name="onT")   # normed oT

    a2a_in = dram.tile([4096, 64], BF16, name="a2a_in")
    a2a_out = dram.tile([4096, 64], BF16, name="a2a_out")

    # =====================================================================
    # Phase A: LN1 stats from xT, hT, projections, gates, tm-transposes
    # =====================================================================
    pA = tc.alloc_tile_pool(name="pA", bufs=1)
    xT = pA.tile([128, DT, T], BF16, name="xT")
    nc.sync.dma_start(xT[:], x_t.ap().rearrange("(dt p) t -> p dt t", p=P))

    rows = tc.alloc_tile_pool(name="rows", bufs=1)
    mu_bf = rows.tile([1, T], BF16, name="mu_bf")
    rstd_bf = rows.tile([1, T], BF16, name="rstd_bf")

    with tc.tile_pool(name="stat_ps", bufs=1, space="PSUM") as stps, \
         tc.tile_pool(name="stat_sb", bufs=2) as stsb:
        ps_s = stps.tile([128, T], F32, name="ps_s")
        ps_q = stps.tile([128, T], F32, name="ps_q")
        for dt in range(DT):
            xsq = stsb.tile([128, T], BF16, name="xsq")
            nc.vector.tensor_tensor(xsq[:], xT[:, dt, :], xT[:, dt, :],
                                    ALU.mult)
            nc.tensor.matmul(ps_s[0:1, :], ones1_sb[:], xT[:, dt, :],
                             start=(dt == 0), stop=(dt == DT - 1))
            nc.tensor.matmul(ps_q[0:1, :], ones1_sb[:], xsq[:],
                             start=(dt == 0), stop=(dt == DT - 1))
        m32 = stsb.tile([1, T], F32, name="m32")
        nc.scalar.activation(m32[:], ps_s[0:1, :], AF.Identity,
                             scale=1.0 / D)
        q32 = stsb.tile([1, T], F32, name="q32")
        nc.scalar.activation(q32[:], ps_q[0:1, :], AF.Identity,
                             scale=1.0 / D)
        var = stsb.tile([1, T], F32, name="var")
        nc.vector.tensor_tensor(var[:], m32[:], m32[:], ALU.mult)
        nc.vector.tensor_tensor(var[:], q32[:], var[:], ALU.subtract)
        lnv = stsb.tile([1, T], F32, name="lnv")
        nc.scalar.activation(lnv[:], var[:], AF.Ln, bias=eps_sb[0:1, 0:1])
        nc.scalar.activation(rstd_bf[:], lnv[:], AF.Exp, scale=-0.5)
        nc.scalar.activation(mu_bf[:], m32[:], AF.Identity)

    hT = pA.tile([128, DT, T], BF16, name="hT")
    with tc.tile_pool(name="mr_ps", bufs=1, space="PSUM") as mrps:
        MU = mrps.tile([128, T], F32, name="MU")
        RSTD = mrps.tile([128, T], F32, name="RSTD")
        nc.tensor.matmul(MU[:], ones_row_sb[:], mu_bf[:],
                         start=True, stop=True)
        nc.tensor.matmul(RSTD[:], ones_row_sb[:], rstd_bf[:],
                         start=True, stop=True)
        for dt in range(DT):
            nc.vector.tensor_tensor(hT[:, dt, :], xT[:, dt, :], MU[:],
                                    ALU.subtract)
            nc.vector.tensor_tensor(hT[:, dt, :], hT[:, dt, :], RSTD[:],
                                    ALU.mult)

        if (d := dump("hT", [128, DT * T], BF16)) is not None:
            nc.sync.dma_start(d.ap().rearrange("p (n f) -> p n f", n=DT),
                              hT[:])

        # projections + gates + token-major transposes, per head-block
        with tc.tile_pool(name="proj_ps", bufs=3, space="PSUM") as pps, \
             tc.tile_pool(name="tr_ps", bufs=1, space="PSUM") as trp, \
             tc.tile_pool(name="pa_sb", bufs=2) as pasb:
            for hb in range(HB):
                vfh = pasb.tile([128, T], BF16, name="vfh")
                ffh = pasb.tile([128, T], BF16, name="ffh")
                for (w_sb, bias, fn, dst) in (
                        (wq_sb, bqp_sb, AF.Silu, qT[:, hb, :]),
                        (wk_sb, bkp_sb, AF.Silu, kT[:, hb, :]),
                        (wv_sb, bvp_sb, AF.Identity, vfh[:]),
                        (wf_sb, bfp_sb, AF.Identity, ffh[:])):
                    bank = pps.tile([128, T], F32, name="projbank")
                    for dt in range(DT):
                        nc.tensor.matmul(bank[:], w_sb[:, dt, hb, :],
                                         hT[:, dt, :],
                                         start=(dt == 0), stop=(dt == DT - 1))
                    nc.scalar.activation(dst, bank[:], fn,
                                         bias=bias[:, hb:hb + 1])
                # PE transposes to token-major  [t, j]
                trA = trp.tile([128, 1024], BF16, name="trA")
                trk, trv = trA[:, 0:512], trA[:, 512:1024]
                trf = trp.tile([128, 512], BF16, name="trf")
                for c in range(NCH):
                    csl = slice(c * 128, (c + 1) * 128)
                    nc.tensor.transpose(trk[:, csl], kT[:, hb, csl],
                                        ident_sb[:])
                    nc.tensor.transpose(trv[:, csl], vfh[:, csl],
                                        ident_sb[:])
                    nc.tensor.transpose(trf[:, csl], ffh[:, csl],
                                        ident_sb[:])
                for c in range(NCH):
                    csl = slice(c * 128, (c + 1) * 128)
                    nc.scalar.copy(k_tm[:, hb, c, :], trk[:, csl])
                    nc.scalar.copy(v_tm[:, hb, c, :], trv[:, csl])
                    # gates: sp = softplus(-f); s = 1 - exp(-sp/8)
                    enf = pasb.tile([128, 128], F32, name="enf")
                    nc.scalar.activation(enf[:], trf[:, csl], AF.Exp,
                                         scale=-1.0)
                    nc.scalar.activation(sp[:, hb, c, :], enf[:], AF.Ln,
                                         bias=1.0)
                    e8 = pasb.tile([128, 128], BF16, name="e8")
                    nc.scalar.activation(e8[:], sp[:, hb, c, :], AF.Exp,
                                         scale=-0.125)
                    nc.vector.tensor_scalar(s_tm[:, hb, c, :], e8[:],
                                            -1.0, 1.0, ALU.mult, ALU.add)

    rows.release()
    pA.release()

    for nm, t_sb in (("qT", qT), ("kT", kT)):
        if (d := dump(nm, [128, HB * T], BF16)) is not None:
            nc.sync.dma_start(d.ap().rearrange("p (n f) -> p n f", n=HB),
                              t_sb[:])
    for nm, t_sb in (("k_tm", k_tm), ("v_tm", v_tm), ("s_tm", s_tm)):
        if (d := dump(nm, [128, HB * NCH * 128], BF16)) is not None:
            nc.sync.dma_start(
                d.ap().rearrange("p (hb c f) -> p hb c f", hb=HB, c=NCH),
                t_sb[:])
    if (d := dump("sp", [128, HB * NCH * 128])) is not None:
        nc.sync.dma_start(
            d.ap().rearrange("p (hb c f) -> p hb c f", hb=HB, c=NCH), sp[:])

    # =====================================================================
    # Phase B: chunked scan — 4 independent head-block chains per chunk
    # =====================================================================
    with tc.tile_pool(name="spsA", bufs=2, space="PSUM") as spsA, \
         tc.tile_pool(name="spsB", bufs=2, space="PSUM") as spsB, \
         tc.tile_pool(name="spsD", bufs=2, space="PSUM") as spsD, \
         tc.tile_pool(name="spsE", bufs=2, space="PSUM") as spsE, \
         tc.tile_pool(name="scan_sb", bufs=3) as ssb, \
         tc.tile_pool(name="state_sb", bufs=1) as stb:
        Kst = stb.tile([128, HB, 64], BF16, name="Kst")   # [2h*64 dk, hb, m]
        Vst = stb.tile([128, HB, 64], BF16, name="Vst")   # [2h*64 s, hb, dv]
        for c in range(NCH):
            csl = slice(c * 128, (c + 1) * 128)
            first = (c == 0)
            for hb in range(HB):
                bankA = spsA.tile([128, 512], F32, name="bankA")
                ps_b = bankA[:, 0:128]
                ps_lc = bankA[:, 128:129]
                ps_lambc = bankA[:, 132:260]
                ps_dv = (bankA[0:64, 260:324], bankA[0:64, 324:388])
                bankB = spsB.tile([128, 512], F32, name="bankB")
                ps_a = (bankB[:, 0:128], bankB[:, 128:256])
                ps_ok = bankB[:, 256:384]
                ps_dk = (bankB[0:64, 384:448], bankB[0:64, 448:512])
                bankD = spsD.tile([128, 1024], BF16, name="bankD")
                ps_pt = (bankD[0:64, 0:128], bankD[0:64, 128:256])
                ps_st = (bankD[0:64, 256:384], bankD[0:64, 384:512])
                ps_lcr = bankD[0:1, 512:640]
                bankE = spsE.tile([128, 512], F32, name="bankE")
                ps_b2 = (bankE[:, 0:128], bankE[:, 128:256])
                ps_o = (bankE[0:64, 256:384], bankE[0:64, 384:512])

                sp_t = sp[:, hb, c, :]
                # cumulative log-decay b = ltriT.T @ (-0.125 sp)  (f32)
                nc.tensor.matmul(ps_b, ltriT_sb[:], sp_t,
                                 start=True, stop=True)
                nc.tensor.matmul(ps_lc, sp_t, onescol_sb[:],
                                 start=True, stop=True)
                lam = ssb.tile([128, 128], BF16, name="lam")
                nc.scalar.activation(lam[:], ps_b, AF.Exp)
                en = ssb.tile([128, 128], BF16, name="en")
                nc.scalar.activation(en[:], ps_b, AF.Exp, scale=-1.0)
                lamCT = ssb.tile([128, 1], F32, name="lamCT")
                nc.scalar.activation(lamCT[:], ps_lc, AF.Exp)
                lamCT16 = ssb.tile([128, 1], BF16, name="lamCT16")
                nc.scalar.activation(lamCT16[:], ps_lc, AF.Exp)
                nc.tensor.transpose(ps_lcr, lamCT16[:], ident_sb[:])
                lamCr = ssb.tile([1, 128], BF16, name="lamCr")
                nc.scalar.copy(lamCr[:], ps_lcr)
                nc.tensor.matmul(ps_lambc, ones_row_sb[:], lamCr[:],
                                 start=True, stop=True)

                s_til = ssb.tile([128, 128], BF16, name="s_til")
                nc.vector.tensor_tensor(s_til[:], s_tm[:, hb, c, :], en[:],
                                        ALU.mult)
                s2 = ssb.tile([128, 128], BF16, name="s2")
                nc.vector.tensor_tensor(s2[:], s_til[:], ps_lambc, ALU.mult)

                am = ssb.tile([128, 256], BF16, name="am")
                for h in range(2):
                    hs = slice(h * 64, (h + 1) * 64)
                    nc.tensor.matmul(ps_a[h], kT[hs, hb, csl],
                                     qT[hs, hb, csl], start=True, stop=True)
                    nc.vector.tensor_tensor(am[:, h * 128:(h + 1) * 128],
                                            ps_a[h], cmask_sb[:], ALU.mult)
                for h in range(2):
                    hs = slice(h * 64, (h + 1) * 64)
                    oks = ps_ok[:, h * 64:(h + 1) * 64]
                    if not first:
                        nc.tensor.matmul(oks, qT[hs, hb, csl],
                                         Kst[hs, hb, :],
                                         start=True, stop=False)
                    nc.tensor.matmul(oks, am[:, h * 128:(h + 1) * 128],
                                     s_til[:, hs], start=first, stop=True)
                # softmax over slots (per head); true logits 0.125*lam*ps_ok
                oksc = ssb.tile([128, 128], F32, name="oksc")
                nc.vector.tensor_tensor(oksc[:], ps_ok, lam[:], ALU.mult)
                ex = ssb.tile([128, 128], BF16, name="ex")
                rsum = ssb.tile([128, 2], F32, name="rsum")
                for h in range(2):
                    hs = slice(h * 64, (h + 1) * 64)
                    nc.scalar.activation(ex[:, hs], oksc[:, hs], AF.Exp,
                                         scale=0.125,
                                         accum_out=rsum[:, h:h + 1])
                rcp = ssb.tile([128, 2], F32, name="rcp")
                nc.vector.reciprocal(rcp[:], rsum[:])
                pl = ssb.tile([128, 128], BF16, name="pl")
                nc.vector.tensor_tensor(pl[:], ex[:], lam[:], ALU.mult)
                nc.vector.tensor_tensor(
                    pl[:].rearrange("p (h s) -> p h s", h=2),
                    pl[:].rearrange("p (h s) -> p h s", h=2),
                    rcp[:].rearrange("p (h o) -> p h o", h=2)
                        .to_broadcast([128, 2, 64]),
                    ALU.mult)

                # transposes: plT, s_tilT  [2h*64 s, 128 t]
                plT = ssb.tile([128, 128], BF16, name="plT")
                s_tilT = ssb.tile([128, 128], BF16, name="s_tilT")
                for h in range(2):
                    hs = slice(h * 64, (h + 1) * 64)
                    nc.tensor.transpose(ps_pt[h], pl[:, hs], ident_sb[:])
                    nc.tensor.transpose(ps_st[h], s_til[:, hs], ident_sb[:])
                for h in range(2):
                    hs = slice(h * 64, (h + 1) * 64)
                    nc.scalar.copy(plT[hs, :], ps_pt[h])
                    nc.scalar.copy(s_tilT[hs, :], ps_st[h])

                b2m = ssb.tile([128, 256], BF16, name="b2m")
                for h in range(2):
                    hs = slice(h * 64, (h + 1) * 64)
                    nc.tensor.matmul(ps_b2[h], s_tilT[hs, :], plT[hs, :],
                                     start=True, stop=True)
                    nc.vector.tensor_tensor(b2m[:, h * 128:(h + 1) * 128],
                                            ps_b2[h], cmask_sb[:], ALU.mult)
                for h in range(2):
                    hs = slice(h * 64, (h + 1) * 64)
                    if not first:
                        nc.tensor.matmul(ps_o[h], Vst[hs, hb, :], plT[hs, :],
                                         start=True, stop=False)
                    nc.tensor.matmul(ps_o[h], v_tm[:, hb, c, hs],
                                     b2m[:, h * 128:(h + 1) * 128],
                                     start=first, stop=True)
                    nc.tensor.matmul(ps_dk[h], k_tm[:, hb, c, hs],
                                     s2[:, hs], start=True, stop=True)
                    nc.tensor.matmul(ps_dv[h], s2[:, hs],
                                     v_tm[:, hb, c, hs],
                                     start=True, stop=True)
                    if first:
                        nc.vector.tensor_copy(Kst[hs, hb, :], ps_dk[h])
                        nc.vector.tensor_copy(Vst[hs, hb, :], ps_dv[h])
                    else:
                        nc.vector.tensor_tensor(
                            Kst[hs, hb, :], Kst[hs, hb, :],
                            ps_lambc[hs, hs], ALU.mult)
                        nc.vector.tensor_tensor(Kst[hs, hb, :],
                                                Kst[hs, hb, :],
                                                ps_dk[h], ALU.add)
                        nc.vector.tensor_scalar(Vst[hs, hb, :],
                                                Vst[hs, hb, :],
                                                lamCT[hs, 0:1], None,
                                                ALU.mult)
                        nc.vector.tensor_tensor(Vst[hs, hb, :],
                                                Vst[hs, hb, :],
                                                ps_dv[h], ALU.add)

                nc.scalar.copy(onT[0:64, hb, csl], ps_o[0])
                nc.scalar.copy(onT[64:128, hb, csl], ps_o[1])

    # =====================================================================
    # Phase C: per-head RMS over dv, then pair-local AllToAll
    # =====================================================================
    with tc.tile_pool(name="rms_ps", bufs=2, space="PSUM") as rps, \
         tc.tile_pool(name="rms_sb", bufs=2) as rsb:
        for hb in range(HB):
            sqo = rsb.tile([128, T], BF16, name="sqo")
            nc.vector.tensor_tensor(sqo[:], onT[:, hb, :], onT[:, hb, :],
                                    ALU.mult)
            ps_ss = rps.tile([128, T], F32, name="ps_ss")
            nc.tensor.matmul(ps_ss[:], bd128_sb[:], sqo[:],
                             start=True, stop=True)
            lnms = rsb.tile([128, T], F32, name="lnms")
            nc.scalar.activation(lnms[:], ps_ss[:], AF.Ln,
                                 bias=eps_sb[:], scale=1.0 / DV)
            rro = rsb.tile([128, T], F32, name="rro")
            nc.scalar.activation(rro[:], lnms[:], AF.Exp, scale=-0.5)
            nc.vector.tensor_tensor(onT[:, hb, :], onT[:, hb, :], rro[:],
                                    ALU.mult)

    if (d := dump("onT", [128, HB * T], BF16)) is not None:
        nc.sync.dma_start(d.ap().rearrange("p (n f) -> p n f", n=HB), onT[:])

    # head-sharded -> token-sharded redistribution. Core d's tail tokens
    # are, for every batch g, the in-batch strip [d*64, (d+1)*64): so the
    # slice this core sends to d is its 512 head-dims x that 64-token
    # strip, and every A2A slice carries useful data.
    for hb in range(HB):
        nc.sync.dma_start(
            a2a_in[:].rearrange("(dst hb p) t -> p hb dst t",
                                p=P, hb=HB)[:, hb],
            onT[:, hb, :].rearrange("p (dst t) -> p dst t", dst=8))
    nc.gpsimd.collective_compute("AllToAll", ALU.bypass, replica_groups=RG,
                                 ins=[a2a_in.opt()], outs=[a2a_out.opt()])

    # =====================================================================
    # Phase D tail: out-proj + residual + LN2 + MLP on 256 tokens
    # =====================================================================
    with tc.tile_pool(name="tail_keep", bufs=1) as tkb, \
         tc.tile_pool(name="tail_sb", bufs=2) as tsb:
        x2 = tkb.tile([128, 2, D], F32, name="x2")
        nc.sync.dma_start(x2[:],
                          x_res.ap().rearrange("(n p) d -> p n d", p=P))
        # a2a_out rows are [src=(g,i), hb, p]; tail token order is (g, t64)
        ofT = tkb.tile([128, DT, TAIL], BF16, name="ofT")
        for g in range(4):
            nc.sync.dma_start(
                ofT[:, :, g * 64:(g + 1) * 64],
                a2a_out[:].rearrange(
                    "(g i hb p) t -> p g (i hb) t", g=4, i=2, hb=HB,
                    p=P)[:, g])

        h2s = [tkb.tile([128, DT * 128], BF16, name=f"h2s{i}")
               for i in range(2)]
        with tc.tile_pool(name="op_ps", bufs=1, space="PSUM") as ops, \
             tc.tile_pool(name="h2_ps", bufs=2, space="PSUM") as h2ps:
            opb = [ops.tile([128, 512], F32, name=f"opb{i}")
                   for i in range(4)]
            for tt2 in range(2):
                t2sl = slice(tt2 * 128, (tt2 + 1) * 128)
                for jt in range(DT):
                    for nb in range(2):
                        nc.tensor.matmul(opb[tt2 * 2 + nb],
                                         ofT[:, jt, t2sl],
                                         wo_sb[:, jt,
                                               nb * 512:(nb + 1) * 512],
                                         start=(jt == 0),
                                         stop=(jt == DT - 1))
                for nb in range(2):
                    nsl = slice(nb * 512, (nb + 1) * 512)
                    nc.vector.tensor_tensor(x2[:, tt2, nsl],
                                            opb[tt2 * 2 + nb],
                                            x2[:, tt2, nsl], ALU.add)
            if (d := dump("x2", [128, 2 * D])) is not None:
                nc.sync.dma_start(d.ap().rearrange("p (n f) -> p n f", n=2),
                                  x2[:])

            # LN2 + transpose-produce h2s[tt2] = h2.T slabs
            for tt2 in range(2):
                x2t = x2[:, tt2, :]
                ssum = tsb.tile([128, 1], F32, name="ssum2")
                nc.vector.tensor_reduce(ssum[:], x2t, AX.X, ALU.add)
                sq = tsb.tile([128, D], BF16, name="sq2")
                ssq = tsb.tile([128, 1], F32, name="ssq2")
                nc.scalar.activation(sq[:], x2t, AF.Square, accum_out=ssq[:])
                mu = tsb.tile([128, 1], F32, name="mu2")
                nc.vector.tensor_scalar_mul(mu[:], ssum[:], 1.0 / D)
                var = tsb.tile([128, 1], F32, name="var2")
                nc.vector.tensor_tensor(var[:], mu[:], mu[:], ALU.mult)
                ex2 = tsb.tile([128, 1], F32, name="ex22")
                nc.vector.tensor_scalar_mul(ex2[:], ssq[:], 1.0 / D)
                nc.vector.tensor_tensor(var[:], ex2[:], var[:], ALU.subtract)
                lnv2 = tsb.tile([128, 1], F32, name="lnv2")
                nc.scalar.activation(lnv2[:], var[:], AF.Ln, bias=eps_sb[:])
                r2 = tsb.tile([128, 1], F32, name="r2")
                nc.scalar.activation(r2[:], lnv2[:], AF.Exp, scale=-0.5)
                nrmu = tsb.tile([128, 1], F32, name="nrmu2")
                nc.vector.tensor_tensor(nrmu[:], r2[:], mu[:], ALU.mult)
                nc.vector.tensor_scalar_mul(nrmu[:], nrmu[:], -1.0)
                h2t = tsb.tile([128, D], BF16, name="h2t")
                nc.scalar.activation(h2t[:], x2t, AF.Identity,
                                     bias=nrmu[:], scale=r2[:])
                if (dd := dump(f"h2tm{tt2}", [128, D], BF16)) is not None:
                    nc.sync.dma_start(dd.ap(), h2t[:])
                tr2 = h2ps.tile([128, 1024], BF16, name="tr2")
                for dt in range(DT):
                    nc.tensor.transpose(tr2[:, dt * 128:(dt + 1) * 128],
                                        h2t[:, dt * 128:(dt + 1) * 128],
                                        ident_sb[:])
                nc.scalar.copy(h2s[tt2][:], tr2[:])

        if (d := dump("h2T", [128, 2 * DT * 128], BF16)) is not None:
            nc.sync.dma_start(d.ap().rearrange("p (i f) -> p i f", i=2)[:, 0],
                              h2s[0][:])
            nc.sync.dma_start(d.ap().rearrange("p (i f) -> p i f", i=2)[:, 1],
                              h2s[1][:])

        # MLP1 in m-quarters: y1 = h2 @ W1 + b1, gelu, PE-transpose to zT
        zs = [tkb.tile([128, MLP], BF16, name=f"zs{i}") for i in range(2)]
        with tc.tile_pool(name="mlp1_ps", bufs=1, space="PSUM") as m1ps, \
             tc.tile_pool(name="zt_ps", bufs=2, space="PSUM") as ztps, \
             tc.tile_pool(name="w1s", bufs=3) as w1s, \
             tc.tile_pool(name="z_sb", bufs=3) as zsb:
            y1b = [m1ps.tile([128, 512], F32, name=f"y1b{i}")
                   for i in range(4)]
            for mh in range(4):
                mhsl = slice(mh * 1024, (mh + 1) * 1024)
                for dt in range(DT):
                    w1d = w1s.tile([128, 1024], BF16, name="w1d")
                    nc.sync.dma_start(
                        w1d[:], w1.ap().rearrange(
                            "p (dt m) -> p dt m", dt=DT)[:, dt, mhsl])
                    for tt2 in range(2):
                        t2sl = slice(tt2 * 128, (tt2 + 1) * 128)
                        for mc in range(2):
                            bank = y1b[tt2 * 2 + mc]
                            if dt == 0:
                                nc.tensor.matmul(
                                    bank, ones_row_sb[:],
                                    b1row_sb[0:1,
                                             mh * 1024 + mc * 512:
                                             mh * 1024 + (mc + 1) * 512],
                                    start=True, stop=False)
                            nc.tensor.matmul(
                                bank, h2s[tt2][:, dt * 128:(dt + 1) * 128],
                                w1d[:, mc * 512:(mc + 1) * 512],
                                start=False, stop=(dt == DT - 1))
                for tt2 in range(2):
                    for mc in range(2):
                        zt_sb = zsb.tile([128, 512], BF16, name="zt_sb")
                        nc.scalar.activation(zt_sb[:], y1b[tt2 * 2 + mc],
                                             AF.Gelu)
                        ztr = ztps.tile([128, 512], BF16, name="ztr")
                        for q in range(4):
                            nc.tensor.transpose(
                                ztr[:, q * 128:(q + 1) * 128],
                                zt_sb[:, q * 128:(q + 1) * 128],
                                ident_sb[:])
                        msl = slice((mh * 2 + mc) * 512,
                                    (mh * 2 + mc + 1) * 512)
                        nc.scalar.copy(zs[tt2][:, msl], ztr[:])

        if (d := dump("zT", [128, 2 * MLP], BF16)) is not None:
            nc.sync.dma_start(d.ap().rearrange("p (i f) -> p i f", i=2)[:, 0],
                              zs[0][:])
            nc.sync.dma_start(d.ap().rearrange("p (i f) -> p i f", i=2)[:, 1],
                              zs[1][:])

        # MLP2: y2 = z @ W2, accumulate over mt into 4 resident banks
        with tc.tile_pool(name="mlp2_ps", bufs=1, space="PSUM") as m2ps, \
             tc.tile_pool(name="w2s", bufs=4) as w2s:
            y2b = [m2ps.tile([128, 512], F32, name=f"y2b{i}")
                   for i in range(4)]
            MT = MLP // 128
            for mt in range(MT):
                w2t = w2s.tile([128, D], BF16, name="w2t")
                nc.sync.dma_start(
                    w2t[:], w2.ap().rearrange(
                        "p (mt d) -> p mt d", mt=MT)[:, mt, :])
                for tt2 in range(2):
                    for nb in range(2):
                        nc.tensor.matmul(
                            y2b[tt2 * 2 + nb],
                            zs[tt2][:, mt * 128:(mt + 1) * 128],
                            w2t[:, nb * 512:(nb + 1) * 512],
                            start=(mt == 0), stop=(mt == MT - 1))
            for tt2 in range(2):
                for nb in range(2):
                    nsl = slice(nb * 512, (nb + 1) * 512)
                    ys = tsb.tile([128, 512], F32, name="ys")
                    nc.vector.tensor_tensor(ys[:], y2b[tt2 * 2 + nb],
                                            x2[:, tt2, nsl], ALU.add)
                    nc.sync.dma_start(
                        y_out.ap().rearrange("(n p) d -> p n d", p=P)
                        [:, tt2, nsl], ys[:])

    for pool in (dram, persist, const):
        pool.release()


def _build():
    nc = bacc.Bacc("TRN2", target_bir_lowering=False, debug=False,
                   num_devices=N_CORES)

    def din(name, shape, dt=BF16):
        return nc.dram_tensor(name, shape, dt, kind="ExternalInput")

    io = dict(
        x_t=din("x_t", [D, T]),
        x_res=din("x_res", [TAIL, D], F32),
        wq=din("wq", [128, 4096]), wk=din("wk", [128, 4096]),
        wv=din("wv", [128, 4096]), wf=din("wf", [128, 4096]),
        bqp=din("bqp", [128, HB], F32), bkp=din("bkp", [128, HB], F32),
        bvp=din("bvp", [128, HB], F32), bfp=din("bfp", [128, HB], F32),
        wo=din("wo", [128, DT * D]),
        w1=din("w1", [128, DT * MLP]),
        b1row=din("b1row", [1, MLP]),
        w2=din("w2", [128, (MLP // 128) * D]),
        ltriT=din("ltriT", [128, 128], F32),
        onescol=din("onescol", [128, 1], F32),
        onescol1=din("onescol1", [128, 1]),
        cmask=din("cmask", [128, 128]),
        ident=din("ident", [128, 128]),
        bd128=din("bd128", [128, 128]),
        ones_row=din("ones_row", [1, 128]),
        y_out=nc.dram_tensor("y_out", [TAIL, D], F32, kind="ExternalOutput"),
    )

    dbg = [s for s in os.environ.get("GSA_DEBUG", "").split(",") if s]
    dbg_outs = {}

    def dump(name, shape, dt=F32):
        if name in dbg:
            t = nc.dram_tensor("dbg_" + name, shape, dt,
                               kind="ExternalOutput")
            dbg_outs[name] = t
            return t
        return None

    io["dump"] = dump
    with tile.TileContext(nc) as tcx:
        _emit(nc, tcx, io)
    nc.compile()
    return nc, sorted(dbg_outs)


def _host_prep(inputs):
    """Fold norms/biases into weights; build per-core in_maps."""
    f32 = np.float32
    bf16 = ml_dtypes.bfloat16
    x = np.asarray(inputs["hidden_states"], f32)           # [B, T, D]
    ln1_w = np.asarray(inputs["ln1_w"], f32)
    ln1_b = np.asarray(inputs["ln1_b"], f32)
    ln2_w = np.asarray(inputs["ln2_w"], f32)
    ln2_b = np.asarray(inputs["ln2_b"], f32)
    gnorm = np.asarray(inputs["gnorm_w"], f32)
    Wq = np.asarray(inputs["Wq"], f32) * ln1_w[:, None]
    Wk = np.asarray(inputs["Wk"], f32) * ln1_w[:, None]
    Wv = np.asarray(inputs["Wv"], f32) * ln1_w[:, None]
    Wf = np.asarray(inputs["Wf"], f32) * ln1_w[:, None]
    bq = ln1_b @ np.asarray(inputs["Wq"], f32)
    bk = ln1_b @ np.asarray(inputs["Wk"], f32)
    bv = ln1_b @ np.asarray(inputs["Wv"], f32)
    bf_ = ln1_b @ np.asarray(inputs["Wf"], f32)
    Wo = np.asarray(inputs["Wo"], f32) * np.tile(gnorm, H)[:, None]
    W1 = np.asarray(inputs["W1"], f32) * ln2_w[:, None]
    b1 = np.asarray(inputs["b1"], f32) + ln2_b @ np.asarray(inputs["W1"], f32)
    W2 = np.asarray(inputs["W2"], f32)
    b2 = np.asarray(inputs["b2"], f32)

    tri = np.tril(np.ones((128, 128), f32))  # [t, tau] tau<=t
    common = dict(
        ltriT=np.ascontiguousarray((-0.125 * tri).T),        # [tau, t]
        onescol=np.full((128, 1), -0.125, f32),
        onescol1=np.ones((128, 1), bf16),
        cmask=np.ascontiguousarray(tri.T.astype(bf16)),      # [tau, t]
        ident=np.eye(128, dtype=bf16),
        bd128=np.kron(np.eye(2, dtype=f32),
                      np.ones((64, 64), f32)).astype(bf16),
        ones_row=np.ones((1, 128), bf16),
        w1=np.ascontiguousarray(
            W1.reshape(DT, 128, MLP).transpose(1, 0, 2)
            .reshape(128, DT * MLP).astype(bf16)),
        b1row=np.ascontiguousarray(b1.reshape(1, MLP).astype(bf16)),
        w2=np.ascontiguousarray(
            W2.reshape(MLP // 128, 128, D).transpose(1, 0, 2)
            .reshape(128, (MLP // 128) * D).astype(bf16)),
        wo=np.ascontiguousarray(
            Wo.reshape(DT, 128, D).transpose(1, 0, 2)
            .reshape(128, DT * D).astype(bf16)),
    )
    in_maps = []
    for r in range(N_CORES):
        g, half = r // 2, r % 2
        jsl = slice(half * 512, half * 512 + 512)  # 8 heads = 512 cols
        m = dict(common)
        m["x_t"] = np.ascontiguousarray(x[g].T.astype(bf16))
        m["x_res"] = np.ascontiguousarray(
            np.concatenate([x[gg, r * 64:(r + 1) * 64] for gg in range(B)])
            + b2[None, :])
        for nm, W in (("wq", Wq), ("wk", Wk), ("wv", Wv), ("wf", Wf)):
            m[nm] = np.ascontiguousarray(
                W[:, jsl].reshape(DT, 128, HB, 128)
                .transpose(1, 0, 2, 3).reshape(128, 4096).astype(bf16))
        for nm, bvec in (("bqp", bq), ("bkp", bk), ("bvp", bv),
                         ("bfp", bf_)):
            m[nm] = np.ascontiguousarray(
                bvec[jsl].reshape(HB, 128).T.astype(f32))
        in_maps.append(m)
    return in_maps


def kernel(**inputs):
    if "nc" not in _cache:
        _cache["nc"], _cache["dbg"] = _build()
    nc = _cache["nc"]
    in_maps = _host_prep(inputs)
    res = run_bass_kernel_spmd(nc, in_maps, core_ids=list(range(N_CORES)),
                               trace=bool(os.environ.get("GSA_TRACE")))
    _cache["last_results"] = res
    out = np.zeros((B, T, D), np.float32)
    for r in range(N_CORES):
        yr = res.results[r]["y_out"]
        for g in range(B):
            out[g, r * 64:(r + 1) * 64, :] = yr[g * 64:(g + 1) * 64]
    return out


# revision 54
# speedup vs baseline: 1.5488x; 1.2282x over previous
"""GSA video block kernel for 8 TRN2 NeuronCores — batch-pair sharding.

Cores pair up: group g = {2g, 2g+1} owns batch g end-to-end. Within a
group each core computes 8 heads (4 head-blocks of 2) of the gated-slot
attention over the batch's 512 tokens; a pair-local AllToAll then
redistributes head outputs to token halves for the fused out-proj + LN2
+ MLP tail (256 tokens per core, full MLP weights streamed from HBM).

The T=512 scan runs chunk-parallel (C=128) exactly as the reference:
intra-chunk causal-masked matmuls with per-slot decay, inter-chunk via
carried states K[DK,M] / V[M,DV] per head.

All transposes run on the PE array (identity matmul) — no DMA
transposes. Positive-value reciprocals use exp(-ln(x)) on the scalar
engine instead of DVE reciprocal.
"""

import os
import sys

import numpy as np
import ml_dtypes

if "/opt/trn_rl_repo" not in sys.path:
    sys.path.insert(0, "/opt/trn_rl_repo")

import concourse.bass as bass  # noqa: E402
import concourse.mybir as mybir  # noqa: E402
import concourse.tile as tile  # noqa: E402
from concourse import bacc  # noqa: E402
from concourse.bass_utils import run_bass_kernel_spmd  # noqa: E402

BF16 = mybir.dt.bfloat16
F32 = mybir.dt.float32
AF = mybir.ActivationFunctionType
ALU = mybir.AluOpType
AX = mybir.AxisListType

B, T, D = 4, 512, 1024
H, DK, DV, M = 16, 64, 64, 64
MLP = 4096
EPS = 1e-6

N_CORES = 8
C = 128                    # scan chunk length
NCH = T // C               # chunks per batch = 4
HB = 4                     # head-blocks per core (2 heads each)
TAIL = 256                 # tokens per core in the tail
DT = D // 128              # 8 d tiles
RG = [list(range(N_CORES))]

_cache = {}


def _emit(nc, tc, io):
    x_t, x_res = io["x_t"], io["x_res"]
    wq, wk, wv, wf = io["wq"], io["wk"], io["wv"], io["wf"]
    bqp, bkp, bvp, bfp = io["bqp"], io["bkp"], io["bvp"], io["bfp"]
    wo, w1, b1row, w2 = io["wo"], io["w1"], io["b1row"], io["w2"]
    ltriT, onescol, onescol1 = io["ltriT"], io["onescol"], io["onescol1"]
    cmask, ident, bd128, ones_row = (io["cmask"], io["ident"], io["bd128"],
                                     io["ones_row"])
    y_out, dump = io["y_out"], io["dump"]
    P = 128

    const = tc.alloc_tile_pool(name="const", bufs=1)
    persist = tc.alloc_tile_pool(name="persist", bufs=1)
    dram = tc.alloc_tile_pool(name="dram", bufs=1, space="DRAM")

    # ---- warmup collective (prepay ncfw handshake) -----------------------
    wa_in = dram.tile([8, 128], BF16, name="wa_in")
    wa_out = dram.tile([8, 128], BF16, name="wa_out")
    nc.gpsimd.collective_compute("AllToAll", ALU.bypass, replica_groups=RG,
                                 ins=[wa_in.opt()], outs=[wa_out.opt()])

    # ---- constants into SBUF --------------------------------------------
    def cload(ap, shape, dt, name):
        t = const.tile(shape, dt, name=name)
        nc.sync.dma_start(t[:], ap)
        return t

    ltriT_sb = cload(ltriT.ap(), [128, 128], F32, "ltriT")
    onescol_sb = cload(onescol.ap(), [128, 1], F32, "onescol")
    ones1_sb = cload(onescol1.ap(), [128, 1], BF16, "ones1")
    cmask_sb = cload(cmask.ap(), [128, 128], BF16, "cmask")
    ident_sb = cload(ident.ap(), [128, 128], BF16, "ident")
    bd128_sb = cload(bd128.ap(), [128, 128], BF16, "bd128")
    ones_row_sb = cload(ones_row.ap(), [1, 128], BF16, "ones_row")
    bqp_sb = cload(bqp.ap(), [128, HB], F32, "bqp")
    bkp_sb = cload(bkp.ap(), [128, HB], F32, "bkp")
    bvp_sb = cload(bvp.ap(), [128, HB], F32, "bvp")
    bfp_sb = cload(bfp.ap(), [128, HB], F32, "bfp")
    b1row_sb = const.tile([1, MLP], BF16, name="b1row")
    eps_sb = const.tile([128, 1], F32)
    nc.vector.memset(eps_sb[:], EPS)

    # xT first on the DMA queue: stats need it before any weights
    pA = tc.alloc_tile_pool(name="pA", bufs=1)
    xT = pA.tile([128, DT, T], BF16, name="xT")
    nc.sync.dma_start(xT[:], x_t.ap().rearrange("(dt p) t -> p dt t", p=P))

    wq_sb = const.tile([128, DT, HB, 128], BF16)
    nc.sync.dma_start(wq_sb[:], wq.ap().rearrange(
        "p (dt hb j) -> p dt hb j", dt=DT, hb=HB))
    wk_sb = const.tile([128, DT, HB, 128], BF16)
    nc.sync.dma_start(wk_sb[:], wk.ap().rearrange(
        "p (dt hb j) -> p dt hb j", dt=DT, hb=HB))
    wv_sb = const.tile([128, DT, HB, 128], BF16)
    nc.sync.dma_start(wv_sb[:], wv.ap().rearrange(
        "p (dt hb j) -> p dt hb j", dt=DT, hb=HB))
    wf_sb = const.tile([128, DT, HB, 128], BF16)
    nc.sync.dma_start(wf_sb[:], wf.ap().rearrange(
        "p (dt hb j) -> p dt hb j", dt=DT, hb=HB))
    # wo_sb is loaded after the scan is emitted (it is only needed in the
    # tail) so its 2MB DMA does not delay xT/weight loads on the queue.
    wo_sb = const.tile([128, DT, D], BF16)

    # ---- persistent activation tensors ----------------------------------
    qT = persist.tile([128, HB, T], BF16, name="qT")     # [2h*64 dk, hb, t]
    kT = persist.tile([128, HB, T], BF16, name="kT")
    k_tm = persist.tile([128, HB, NCH, 128], BF16, name="k_tm")  # [t,hb,c,j]
    v_tm = persist.tile([128, HB, NCH, 128], BF16, name="v_tm")
    sp = persist.tile([128, HB, NCH, 128], F32, name="sp")       # softplus(-f)
    s_tm = persist.tile([128, HB, NCH, 128], BF16, name="s_tm")  # 1-exp(g)
    onT = persist.tile([128, HB, T], BF16, name="onT")   # normed oT

    a2a_in = dram.tile([4096, 64], BF16, name="a2a_in")
    a2a_out = dram.tile([4096, 64], BF16, name="a2a_out")

    # =====================================================================
    # Phase A: LN1 stats from xT, hT, projections, gates, tm-transposes
    # =====================================================================
    rows = tc.alloc_tile_pool(name="rows", bufs=1)
    mu_bf = rows.tile([1, T], BF16, name="mu_bf")
    rstd_bf = rows.tile([1, T], BF16, name="rstd_bf")

    with tc.tile_pool(name="stat_ps", bufs=1, space="PSUM") as stps, \
         tc.tile_pool(name="stat_sb", bufs=2) as stsb:
        ps_s = stps.tile([128, T], F32, name="ps_s")
        ps_q = stps.tile([128, T], F32, name="ps_q")
        for dt in range(DT):
            xsq = stsb.tile([128, T], BF16, name="xsq")
            nc.vector.tensor_tensor(xsq[:], xT[:, dt, :], xT[:, dt, :],
                                    ALU.mult)
            nc.tensor.matmul(ps_s[0:1, :], ones1_sb[:], xT[:, dt, :],
                             start=(dt == 0), stop=(dt == DT - 1))
            nc.tensor.matmul(ps_q[0:1, :], ones1_sb[:], xsq[:],
                             start=(dt == 0), stop=(dt == DT - 1))
        m32 = stsb.tile([1, T], F32, name="m32")
        nc.vector.tensor_scalar_mul(m32[:], ps_s[0:1, :], 1.0 / D)
        q32 = stsb.tile([1, T], F32, name="q32")
        nc.vector.tensor_scalar_mul(q32[:], ps_q[0:1, :], 1.0 / D)
        nc.vector.tensor_copy(mu_bf[:], m32[:])
        var = stsb.tile([1, T], F32, name="var")
        nc.vector.tensor_tensor(var[:], m32[:], m32[:], ALU.mult)
        nc.vector.tensor_tensor(var[:], q32[:], var[:], ALU.subtract)
        lnv = stsb.tile([1, T], F32, name="lnv")
        nc.scalar.activation(lnv[:], var[:], AF.Ln, bias=eps_sb[0:1, 0:1])
        nc.scalar.activation(rstd_bf[:], lnv[:], AF.Exp, scale=-0.5)

    hT = pA.tile([128, DT, T], BF16, name="hT")
    with tc.tile_pool(name="mr_ps", bufs=1, space="PSUM") as mrps:
        MU = mrps.tile([128, T], F32, name="MU")
        RSTD = mrps.tile([128, T], F32, name="RSTD")
        nc.tensor.matmul(MU[:], ones_row_sb[:], mu_bf[:],
                         start=True, stop=True)
        nc.tensor.matmul(RSTD[:], ones_row_sb[:], rstd_bf[:],
                         start=True, stop=True)
        for dt in range(DT):
            nc.vector.tensor_tensor(hT[:, dt, :], xT[:, dt, :], MU[:],
                                    ALU.subtract)
            nc.vector.tensor_tensor(hT[:, dt, :], hT[:, dt, :], RSTD[:],
                                    ALU.mult)

        if (d := dump("hT", [128, DT * T], BF16)) is not None:
            nc.sync.dma_start(d.ap().rearrange("p (n f) -> p n f", n=DT),
                              hT[:])

        # projections + gates + token-major transposes, per head-block
        f_tm = pA.tile([128, HB, NCH, 128], BF16, name="f_tm")
        with tc.tile_pool(name="proj_ps", bufs=3, space="PSUM") as pps, \
             tc.tile_pool(name="tr_ps", bufs=1, space="PSUM") as trp, \
             tc.tile_pool(name="pa_sb", bufs=2) as pasb:
            for hb in range(HB):
                vfh = pasb.tile([128, T], BF16, name="vfh")
                ffh = pasb.tile([128, T], BF16, name="ffh")
                for (w_sb, bias, fn, dst) in (
                        (wq_sb, bqp_sb, AF.Silu, qT[:, hb, :]),
                        (wk_sb, bkp_sb, AF.Silu, kT[:, hb, :]),
                        (wv_sb, bvp_sb, None, vfh[:]),
                        (wf_sb, bfp_sb, None, ffh[:])):
                    bank = pps.tile([128, T], F32, name="projbank")
                    for dt in range(DT):
                        nc.tensor.matmul(bank[:], w_sb[:, dt, hb, :],
                                         hT[:, dt, :],
                                         start=(dt == 0), stop=(dt == DT - 1))
                    if fn is not None:
                        nc.scalar.activation(dst, bank[:], fn,
                                             bias=bias[:, hb:hb + 1])
                    else:
                        nc.vector.tensor_scalar(dst, bank[:],
                                                bias[:, hb:hb + 1], None,
                                                ALU.add)
                # PE transposes to token-major  [t, j]
                trA = trp.tile([128, 1024], BF16, name="trA")
                trk, trv = trA[:, 0:512], trA[:, 512:1024]
                trf = trp.tile([128, 512], BF16, name="trf")
                for c in range(NCH):
                    csl = slice(c * 128, (c + 1) * 128)
                    nc.tensor.transpose(trk[:, csl], kT[:, hb, csl],
                                        ident_sb[:])
                    nc.tensor.transpose(trv[:, csl], vfh[:, csl],
                                        ident_sb[:])
                    nc.tensor.transpose(trf[:, csl], ffh[:, csl],
                                        ident_sb[:])
                for c in range(NCH):
                    csl = slice(c * 128, (c + 1) * 128)
                    nc.vector.tensor_copy(k_tm[:, hb, c, :], trk[:, csl])
                    nc.vector.tensor_copy(v_tm[:, hb, c, :], trv[:, csl])
                    nc.vector.tensor_copy(f_tm[:, hb, c, :], trf[:, csl])

            # gates, batched per activation function to avoid table reloads:
            # sp = softplus(-f) = ln(1 + exp(-f)); s = 1 - exp(-sp/8)
            enf = pA.tile([128, HB, NCH, 128], F32, name="enf")
            for hb in range(HB):
                nc.scalar.activation(enf[:, hb], f_tm[:, hb], AF.Exp,
                                     scale=-1.0)
            for hb in range(HB):
                nc.scalar.activation(sp[:, hb], enf[:, hb], AF.Ln, bias=1.0)
            e8 = pA.tile([128, HB, NCH, 128], BF16, name="e8")
            for hb in range(HB):
                nc.scalar.activation(e8[:, hb], sp[:, hb], AF.Exp,
                                     scale=-0.125)
            for hb in range(HB):
                nc.vector.tensor_scalar(s_tm[:, hb], e8[:, hb],
                                        -1.0, 1.0, ALU.mult, ALU.add)

    rows.release()
    pA.release()

    for nm, t_sb in (("qT", qT), ("kT", kT)):
        if (d := dump(nm, [128, HB * T], BF16)) is not None:
            nc.sync.dma_start(d.ap().rearrange("p (n f) -> p n f", n=HB),
                              t_sb[:])
    for nm, t_sb in (("k_tm", k_tm), ("v_tm", v_tm), ("s_tm", s_tm)):
        if (d := dump(nm, [128, HB * NCH * 128], BF16)) is not None:
            nc.sync.dma_start(
                d.ap().rearrange("p (hb c f) -> p hb c f", hb=HB, c=NCH),
                t_sb[:])
    if (d := dump("sp", [128, HB * NCH * 128])) is not None:
        nc.sync.dma_start(
            d.ap().rearrange("p (hb c f) -> p hb c f", hb=HB, c=NCH), sp[:])

    # =====================================================================
    # Phase B: chunked scan — 4 independent head-block chains per chunk
    # =====================================================================
    with tc.tile_pool(name="spsA", bufs=2, space="PSUM") as spsA, \
         tc.tile_pool(name="spsB", bufs=2, space="PSUM") as spsB, \
         tc.tile_pool(name="spsD", bufs=2, space="PSUM") as spsD, \
         tc.tile_pool(name="spsE", bufs=2, space="PSUM") as spsE, \
         tc.tile_pool(name="scan_sb", bufs=3) as ssb, \
         tc.tile_pool(name="state_sb", bufs=1) as stb:
        Kst = stb.tile([128, HB, 64], BF16, name="Kst")   # [2h*64 dk, hb, m]
        Vst = stb.tile([128, HB, 64], BF16, name="Vst")   # [2h*64 s, hb, dv]
        for c in range(NCH):
            csl = slice(c * 128, (c + 1) * 128)
            first = (c == 0)
            for hb in range(HB):
                bankA = spsA.tile([128, 512], F32, name="bankA")
                ps_b = bankA[:, 0:128]
                ps_lc = bankA[:, 128:129]
                ps_lambc = bankA[:, 132:260]
                ps_dv = (bankA[0:64, 260:324], bankA[0:64, 324:388])
                bankB = spsB.tile([128, 512], F32, name="bankB")
                ps_a = (bankB[:, 0:128], bankB[:, 128:256])
                ps_ok = bankB[:, 256:384]
                ps_dk = (bankB[0:64, 384:448], bankB[0:64, 448:512])
                bankD = spsD.tile([128, 1024], BF16, name="bankD")
                ps_pt = (bankD[0:64, 0:128], bankD[0:64, 128:256])
                ps_st = (bankD[0:64, 256:384], bankD[0:64, 384:512])
                ps_lcr = bankD[0:1, 512:640]
                bankE = spsE.tile([128, 512], F32, name="bankE")
                ps_b2 = (bankE[:, 0:128], bankE[:, 128:256])
                ps_o = (bankE[0:64, 256:384], bankE[0:64, 384:512])

                sp_t = sp[:, hb, c, :]
                # cumulative log-decay b = ltriT.T @ (-0.125 sp)  (f32)
                nc.tensor.matmul(ps_b, ltriT_sb[:], sp_t,
                                 start=True, stop=True)
                nc.tensor.matmul(ps_lc, sp_t, onescol_sb[:],
                                 start=True, stop=True)
                lam = ssb.tile([128, 128], BF16, name="lam")
                nc.scalar.activation(lam[:], ps_b, AF.Exp)
                en = ssb.tile([128, 128], BF16, name="en")
                nc.scalar.activation(en[:], ps_b, AF.Exp, scale=-1.0)
                lamCT = ssb.tile([128, 1], F32, name="lamCT")
                nc.scalar.activation(lamCT[:], ps_lc, AF.Exp)
                lamCT16 = ssb.tile([128, 1], BF16, name="lamCT16")
                nc.scalar.activation(lamCT16[:], ps_lc, AF.Exp)
                nc.tensor.transpose(ps_lcr, lamCT16[:], ident_sb[:])
                lamCr = ssb.tile([1, 128], BF16, name="lamCr")
                nc.vector.tensor_copy(lamCr[:], ps_lcr)
                nc.tensor.matmul(ps_lambc, ones_row_sb[:], lamCr[:],
                                 start=True, stop=True)

                s_til = ssb.tile([128, 128], BF16, name="s_til")
                nc.vector.tensor_tensor(s_til[:], s_tm[:, hb, c, :], en[:],
                                        ALU.mult)
                s2 = ssb.tile([128, 128], BF16, name="s2")
                nc.vector.tensor_tensor(s2[:], s_til[:], ps_lambc, ALU.mult)

                am = ssb.tile([128, 256], BF16, name="am")
                for h in range(2):
                    hs = slice(h * 64, (h + 1) * 64)
                    nc.tensor.matmul(ps_a[h], kT[hs, hb, csl],
                                     qT[hs, hb, csl], start=True, stop=True)
                    nc.vector.tensor_tensor(am[:, h * 128:(h + 1) * 128],
                                            ps_a[h], cmask_sb[:], ALU.mult)
                for h in range(2):
                    hs = slice(h * 64, (h + 1) * 64)
                    oks = ps_ok[:, h * 64:(h + 1) * 64]
                    if not first:
                        nc.tensor.matmul(oks, qT[hs, hb, csl],
                                         Kst[hs, hb, :],
                                         start=True, stop=False)
                    nc.tensor.matmul(oks, am[:, h * 128:(h + 1) * 128],
                                     s_til[:, hs], start=first, stop=True)
                # softmax over slots (per head); true logits 0.125*lam*ps_ok
                oksc = ssb.tile([128, 128], F32, name="oksc")
                nc.vector.tensor_tensor(oksc[:], ps_ok, lam[:], ALU.mult)
                ex = ssb.tile([128, 128], BF16, name="ex")
                rsum = ssb.tile([128, 2], F32, name="rsum")
                for h in range(2):
                    hs = slice(h * 64, (h + 1) * 64)
                    nc.scalar.activation(ex[:, hs], oksc[:, hs], AF.Exp,
                                         scale=0.125,
                                         accum_out=rsum[:, h:h + 1])
                rcp = ssb.tile([128, 2], F32, name="rcp")
                nc.vector.reciprocal(rcp[:], rsum[:])
                pl = ssb.tile([128, 128], BF16, name="pl")
                nc.vector.tensor_tensor(pl[:], ex[:], lam[:], ALU.mult)
                nc.vector.tensor_tensor(
                    pl[:].rearrange("p (h s) -> p h s", h=2),
                    pl[:].rearrange("p (h s) -> p h s", h=2),
                    rcp[:].rearrange("p (h o) -> p h o", h=2)
                        .to_broadcast([128, 2, 64]),
                    ALU.mult)

                # transposes: plT, s_tilT  [2h*64 s, 128 t]
                plT = ssb.tile([128, 128], BF16, name="plT")
                s_tilT = ssb.tile([128, 128], BF16, name="s_tilT")
                for h in range(2):
                    hs = slice(h * 64, (h + 1) * 64)
                    nc.tensor.transpose(ps_pt[h], pl[:, hs], ident_sb[:])
                    nc.tensor.transpose(ps_st[h], s_til[:, hs], ident_sb[:])
                for h in range(2):
                    hs = slice(h * 64, (h + 1) * 64)
                    nc.vector.tensor_copy(plT[hs, :], ps_pt[h])
                    nc.vector.tensor_copy(s_tilT[hs, :], ps_st[h])

                b2m = ssb.tile([128, 256], BF16, name="b2m")
                for h in range(2):
                    hs = slice(h * 64, (h + 1) * 64)
                    nc.tensor.matmul(ps_b2[h], s_tilT[hs, :], plT[hs, :],
                                     start=True, stop=True)
                    nc.vector.tensor_tensor(b2m[:, h * 128:(h + 1) * 128],
                                            ps_b2[h], cmask_sb[:], ALU.mult)
                for h in range(2):
                    hs = slice(h * 64, (h + 1) * 64)
                    if not first:
                        nc.tensor.matmul(ps_o[h], Vst[hs, hb, :], plT[hs, :],
                                         start=True, stop=False)
                    nc.tensor.matmul(ps_o[h], v_tm[:, hb, c, hs],
                                     b2m[:, h * 128:(h + 1) * 128],
                                     start=first, stop=True)
                    nc.tensor.matmul(ps_dk[h], k_tm[:, hb, c, hs],
                                     s2[:, hs], start=True, stop=True)
                    nc.tensor.matmul(ps_dv[h], s2[:, hs],
                                     v_tm[:, hb, c, hs],
                                     start=True, stop=True)
                    if first:
                        nc.vector.tensor_copy(Kst[hs, hb, :], ps_dk[h])
                        nc.vector.tensor_copy(Vst[hs, hb, :], ps_dv[h])
                    else:
                        nc.vector.tensor_tensor(
                            Kst[hs, hb, :], Kst[hs, hb, :],
                            ps_lambc[hs, hs], ALU.mult)
                        nc.vector.tensor_tensor(Kst[hs, hb, :],
                                                Kst[hs, hb, :],
                                                ps_dk[h], ALU.add)
                        nc.vector.tensor_scalar(Vst[hs, hb, :],
                                                Vst[hs, hb, :],
                                                lamCT[hs, 0:1], None,
                                                ALU.mult)
                        nc.vector.tensor_tensor(Vst[hs, hb, :],
                                                Vst[hs, hb, :],
                                                ps_dv[h], ALU.add)

                nc.vector.tensor_copy(onT[0:64, hb, csl], ps_o[0])
                nc.vector.tensor_copy(onT[64:128, hb, csl], ps_o[1])

    # deferred tail-weight loads (queue is idle during the scan)
    nc.sync.dma_start(wo_sb[:], wo.ap().rearrange("p (jt n) -> p jt n",
                                                  jt=DT))
    nc.sync.dma_start(b1row_sb[:], b1row.ap())

    # =====================================================================
    # Phase C: per-head RMS over dv, then pair-local AllToAll
    # =====================================================================
    with tc.tile_pool(name="rms_ps", bufs=4, space="PSUM") as rps, \
         tc.tile_pool(name="rms_sb", bufs=4) as rsb:
        pss, lnms, rro = [], [], []
        for hb in range(HB):
            sqo = rsb.tile([128, T], BF16, name="sqo")
            nc.vector.tensor_tensor(sqo[:], onT[:, hb, :], onT[:, hb, :],
                                    ALU.mult)
            ps_ss = rps.tile([128, T], F32, name="ps_ss")
            nc.tensor.matmul(ps_ss[:], bd128_sb[:], sqo[:],
                             start=True, stop=True)
            pss.append(ps_ss)
        for hb in range(HB):
            t = rsb.tile([128, T], F32, name="lnms")
            nc.scalar.activation(t[:], pss[hb][:], AF.Ln,
                                 bias=eps_sb[:], scale=1.0 / DV)
            lnms.append(t)
        for hb in range(HB):
            t = rsb.tile([128, T], F32, name="rro")
            nc.scalar.activation(t[:], lnms[hb][:], AF.Exp, scale=-0.5)
            rro.append(t)
        for hb in range(HB):
            nc.vector.tensor_tensor(onT[:, hb, :], onT[:, hb, :],
                                    rro[hb][:], ALU.mult)

    if (d := dump("onT", [128, HB * T], BF16)) is not None:
        nc.sync.dma_start(d.ap().rearrange("p (n f) -> p n f", n=HB), onT[:])

    # head-sharded -> token-sharded redistribution. Core d's tail tokens
    # are, for every batch g, the in-batch strip [d*64, (d+1)*64): so the
    # slice this core sends to d is its 512 head-dims x that 64-token
    # strip, and every A2A slice carries useful data.
    for hb in range(HB):
        nc.sync.dma_start(
            a2a_in[:].rearrange("(dst hb p) t -> p hb dst t",
                                p=P, hb=HB)[:, hb],
            onT[:, hb, :].rearrange("p (dst t) -> p dst t", dst=8))
    nc.gpsimd.collective_compute("AllToAll", ALU.bypass, replica_groups=RG,
                                 ins=[a2a_in.opt()], outs=[a2a_out.opt()])

    # =====================================================================
    # Phase D tail: out-proj + residual + LN2 + MLP on 256 tokens
    # =====================================================================
    with tc.tile_pool(name="tail_keep", bufs=1) as tkb, \
         tc.tile_pool(name="tail_sb", bufs=2) as tsb:
        x2 = tkb.tile([128, 2, D], F32, name="x2")
        nc.sync.dma_start(x2[:],
                          x_res.ap().rearrange("(n p) d -> p n d", p=P))
        # a2a_out rows are [src=(g,i), hb, p]; tail token order is (g, t64)
        ofT = tkb.tile([128, DT, TAIL], BF16, name="ofT")
        for g in range(4):
            nc.sync.dma_start(
                ofT[:, :, g * 64:(g + 1) * 64],
                a2a_out[:].rearrange(
                    "(g i hb p) t -> p g (i hb) t", g=4, i=2, hb=HB,
                    p=P)[:, g])

        h2s = [tkb.tile([128, DT * 128], BF16, name=f"h2s{i}")
               for i in range(2)]
        with tc.tile_pool(name="op_ps", bufs=1, space="PSUM") as ops, \
             tc.tile_pool(name="h2_ps", bufs=2, space="PSUM") as h2ps:
            opb = [ops.tile([128, 512], F32, name=f"opb{i}")
                   for i in range(4)]
            for tt2 in range(2):
                t2sl = slice(tt2 * 128, (tt2 + 1) * 128)
                for jt in range(DT):
                    for nb in range(2):
                        nc.tensor.matmul(opb[tt2 * 2 + nb],
                                         ofT[:, jt, t2sl],
                                         wo_sb[:, jt,
                                               nb * 512:(nb + 1) * 512],
                                         start=(jt == 0),
                                         stop=(jt == DT - 1))
                for nb in range(2):
                    nsl = slice(nb * 512, (nb + 1) * 512)
                    nc.vector.tensor_tensor(x2[:, tt2, nsl],
                                            opb[tt2 * 2 + nb],
                                            x2[:, tt2, nsl], ALU.add)
            if (d := dump("x2", [128, 2 * D])) is not None:
                nc.sync.dma_start(d.ap().rearrange("p (n f) -> p n f", n=2),
                                  x2[:])

            # LN2 + transpose-produce h2s[tt2] = h2.T slabs
            for tt2 in range(2):
                x2t = x2[:, tt2, :]
                ssum = tsb.tile([128, 1], F32, name="ssum2")
                nc.vector.tensor_reduce(ssum[:], x2t, AX.X, ALU.add)
                sq = tsb.tile([128, D], BF16, name="sq2")
                ssq = tsb.tile([128, 1], F32, name="ssq2")
                nc.scalar.activation(sq[:], x2t, AF.Square, accum_out=ssq[:])
                mu = tsb.tile([128, 1], F32, name="mu2")
                nc.vector.tensor_scalar_mul(mu[:], ssum[:], 1.0 / D)
                var = tsb.tile([128, 1], F32, name="var2")
                nc.vector.tensor_tensor(var[:], mu[:], mu[:], ALU.mult)
                ex2 = tsb.tile([128, 1], F32, name="ex22")
                nc.vector.tensor_scalar_mul(ex2[:], ssq[:], 1.0 / D)
                nc.vector.tensor_tensor(var[:], ex2[:], var[:], ALU.subtract)
                lnv2 = tsb.tile([128, 1], F32, name="lnv2")
                nc.scalar.activation(lnv2[:], var[:], AF.Ln, bias=eps_sb[:])
                r2 = tsb.tile([128, 1], F32, name="r2")
                nc.scalar.activation(r2[:], lnv2[:], AF.Exp, scale=-0.5)
                negmu = tsb.tile([128, 1], F32, name="negmu")
                nc.vector.tensor_scalar_mul(negmu[:], mu[:], -1.0)
                h2t = tsb.tile([128, D], BF16, name="h2t")
                nc.vector.tensor_scalar(h2t[:], x2t, negmu[:], r2[:],
                                        ALU.add, ALU.mult)
                if (dd := dump(f"h2tm{tt2}", [128, D], BF16)) is not None:
                    nc.sync.dma_start(dd.ap(), h2t[:])
                tr2 = h2ps.tile([128, 1024], BF16, name="tr2")
                for dt in range(DT):
                    nc.tensor.transpose(tr2[:, dt * 128:(dt + 1) * 128],
                                        h2t[:, dt * 128:(dt + 1) * 128],
                                        ident_sb[:])
                nc.vector.tensor_copy(h2s[tt2][:], tr2[:])

        if (d := dump("h2T", [128, 2 * DT * 128], BF16)) is not None:
            nc.sync.dma_start(d.ap().rearrange("p (i f) -> p i f", i=2)[:, 0],
                              h2s[0][:])
            nc.sync.dma_start(d.ap().rearrange("p (i f) -> p i f", i=2)[:, 1],
                              h2s[1][:])

        # MLP1 in m-quarters: y1 = h2 @ W1 + b1, gelu, PE-transpose to zT
        zs = [tkb.tile([128, MLP], BF16, name=f"zs{i}") for i in range(2)]
        with tc.tile_pool(name="mlp1_ps", bufs=1, space="PSUM") as m1ps, \
             tc.tile_pool(name="zt_ps", bufs=2, space="PSUM") as ztps, \
             tc.tile_pool(name="w1s", bufs=3) as w1s, \
             tc.tile_pool(name="z_sb", bufs=3) as zsb:
            y1b = [m1ps.tile([128, 512], F32, name=f"y1b{i}")
                   for i in range(4)]
            for mh in range(4):
                mhsl = slice(mh * 1024, (mh + 1) * 1024)
                for dt in range(DT):
                    w1d = w1s.tile([128, 1024], BF16, name="w1d")
                    nc.sync.dma_start(
                        w1d[:], w1.ap().rearrange(
                            "p (dt m) -> p dt m", dt=DT)[:, dt, mhsl])
                    for tt2 in range(2):
                        t2sl = slice(tt2 * 128, (tt2 + 1) * 128)
                        for mc in range(2):
                            bank = y1b[tt2 * 2 + mc]
                            if dt == 0:
                                nc.tensor.matmul(
                                    bank, ones_row_sb[:],
                                    b1row_sb[0:1,
                                             mh * 1024 + mc * 512:
                                             mh * 1024 + (mc + 1) * 512],
                                    start=True, stop=False)
                            nc.tensor.matmul(
                                bank, h2s[tt2][:, dt * 128:(dt + 1) * 128],
                                w1d[:, mc * 512:(mc + 1) * 512],
                                start=False, stop=(dt == DT - 1))
                for tt2 in range(2):
                    for mc in range(2):
                        zt_sb = zsb.tile([128, 512], BF16, name="zt_sb")
                        nc.scalar.activation(zt_sb[:], y1b[tt2 * 2 + mc],
                                             AF.Gelu)
                        ztr = ztps.tile([128, 512], BF16, name="ztr")
                        for q in range(4):
                            nc.tensor.transpose(
                                ztr[:, q * 128:(q + 1) * 128],
                                zt_sb[:, q * 128:(q + 1) * 128],
                                ident_sb[:])
                        msl = slice((mh * 2 + mc) * 512,
                                    (mh * 2 + mc + 1) * 512)
                        nc.vector.tensor_copy(zs[tt2][:, msl], ztr[:])

        if (d := dump("zT", [128, 2 * MLP], BF16)) is not None:
            nc.sync.dma_start(d.ap().rearrange("p (i f) -> p i f", i=2)[:, 0],
                              zs[0][:])
            nc.sync.dma_start(d.ap().rearrange("p (i f) -> p i f", i=2)[:, 1],
                              zs[1][:])

        # MLP2: y2 = z @ W2, accumulate over mt into 4 resident banks
        with tc.tile_pool(name="mlp2_ps", bufs=1, space="PSUM") as m2ps, \
             tc.tile_pool(name="w2s", bufs=4) as w2s:
            y2b = [m2ps.tile([128, 512], F32, name=f"y2b{i}")
                   for i in range(4)]
            MT = MLP // 128
            for mt in range(MT):
                w2t = w2s.tile([128, D], BF16, name="w2t")
                nc.sync.dma_start(
                    w2t[:], w2.ap().rearrange(
                        "p (mt d) -> p mt d", mt=MT)[:, mt, :])
                for tt2 in range(2):
                    for nb in range(2):
                        nc.tensor.matmul(
                            y2b[tt2 * 2 + nb],
                            zs[tt2][:, mt * 128:(mt + 1) * 128],
                            w2t[:, nb * 512:(nb + 1) * 512],
                            start=(mt == 0), stop=(mt == MT - 1))
            for tt2 in range(2):
                for nb in range(2):
                    nsl = slice(nb * 512, (nb + 1) * 512)
                    ys = tsb.tile([128, 512], F32, name="ys")
                    nc.vector.tensor_tensor(ys[:], y2b[tt2 * 2 + nb],
                                            x2[:, tt2, nsl], ALU.add)
                    nc.sync.dma_start(
                        y_out.ap().rearrange("(n p) d -> p n d", p=P)
                        [:, tt2, nsl], ys[:])

    for pool in (dram, persist, const):
        pool.release()


def _build():
    nc = bacc.Bacc("TRN2", target_bir_lowering=False, debug=False,
                   num_devices=N_CORES)

    def din(name, shape, dt=BF16):
        return nc.dram_tensor(name, shape, dt, kind="ExternalInput")

    io = dict(
        x_t=din("x_t", [D, T]),
        x_res=din("x_res", [TAIL, D], F32),
        wq=din("wq", [128, 4096]), wk=din("wk", [128, 4096]),
        wv=din("wv", [128, 4096]), wf=din("wf", [128, 4096]),
        bqp=din("bqp", [128, HB], F32), bkp=din("bkp", [128, HB], F32),
        bvp=din("bvp", [128, HB], F32), bfp=din("bfp", [128, HB], F32),
        wo=din("wo", [128, DT * D]),
        w1=din("w1", [128, DT * MLP]),
        b1row=din("b1row", [1, MLP]),
        w2=din("w2", [128, (MLP // 128) * D]),
        ltriT=din("ltriT", [128, 128], F32),
        onescol=din("onescol", [128, 1], F32),
        onescol1=din("onescol1", [128, 1]),
        cmask=din("cmask", [128, 128]),
        ident=din("ident", [128, 128]),
        bd128=din("bd128", [128, 128]),
        ones_row=din("ones_row", [1, 128]),
        y_out=nc.dram_tensor("y_out", [TAIL, D], F32, kind="ExternalOutput"),
    )

    dbg = [s for s in os.environ.get("GSA_DEBUG", "").split(",") if s]
    dbg_outs = {}

    def dump(name, shape, dt=F32):
        if name in dbg:
            t = nc.dram_tensor("dbg_" + name, shape, dt,
                               kind="ExternalOutput")
            dbg_outs[name] = t
            return t
        return None

    io["dump"] = dump
    with tile.TileContext(nc) as tcx:
        _emit(nc, tcx, io)
    nc.compile()
    return nc, sorted(dbg_outs)


def _host_prep(inputs):
    """Fold norms/biases into weights; build per-core in_maps."""
    f32 = np.float32
    bf16 = ml_dtypes.bfloat16
    x = np.asarray(inputs["hidden_states"], f32)           # [B, T, D]
    ln1_w = np.asarray(inputs["ln1_w"], f32)
    ln1_b = np.asarray(inputs["ln1_b"], f32)
    ln2_w = np.asarray(inputs["ln2_w"], f32)
    ln2_b = np.asarray(inputs["ln2_b"], f32)
    gnorm = np.asarray(inputs["gnorm_w"], f32)
    Wq = np.asarray(inputs["Wq"], f32) * ln1_w[:, None]
    Wk = np.asarray(inputs["Wk"], f32) * ln1_w[:, None]
    Wv = np.asarray(inputs["Wv"], f32) * ln1_w[:, None]
    Wf = np.asarray(inputs["Wf"], f32) * ln1_w[:, None]
    bq = ln1_b @ np.asarray(inputs["Wq"], f32)
    bk = ln1_b @ np.asarray(inputs["Wk"], f32)
    bv = ln1_b @ np.asarray(inputs["Wv"], f32)
    bf_ = ln1_b @ np.asarray(inputs["Wf"], f32)
    Wo = np.asarray(inputs["Wo"], f32) * np.tile(gnorm, H)[:, None]
    W1 = np.asarray(inputs["W1"], f32) * ln2_w[:, None]
    b1 = np.asarray(inputs["b1"], f32) + ln2_b @ np.asarray(inputs["W1"], f32)
    W2 = np.asarray(inputs["W2"], f32)
    b2 = np.asarray(inputs["b2"], f32)

    tri = np.tril(np.ones((128, 128), f32))  # [t, tau] tau<=t
    common = dict(
        ltriT=np.ascontiguousarray((-0.125 * tri).T),        # [tau, t]
        onescol=np.full((128, 1), -0.125, f32),
        onescol1=np.ones((128, 1), bf16),
        cmask=np.ascontiguousarray(tri.T.astype(bf16)),      # [tau, t]
        ident=np.eye(128, dtype=bf16),
        bd128=np.kron(np.eye(2, dtype=f32),
                      np.ones((64, 64), f32)).astype(bf16),
        ones_row=np.ones((1, 128), bf16),
        w1=np.ascontiguousarray(
            W1.reshape(DT, 128, MLP).transpose(1, 0, 2)
            .reshape(128, DT * MLP).astype(bf16)),
        b1row=np.ascontiguousarray(b1.reshape(1, MLP).astype(bf16)),
        w2=np.ascontiguousarray(
            W2.reshape(MLP // 128, 128, D).transpose(1, 0, 2)
            .reshape(128, (MLP // 128) * D).astype(bf16)),
        wo=np.ascontiguousarray(
            Wo.reshape(DT, 128, D).transpose(1, 0, 2)
            .reshape(128, DT * D).astype(bf16)),
    )
    in_maps = []
    for r in range(N_CORES):
        g, half = r // 2, r % 2
        jsl = slice(half * 512, half * 512 + 512)  # 8 heads = 512 cols
        m = dict(common)
        m["x_t"] = np.ascontiguousarray(x[g].T.astype(bf16))
        m["x_res"] = np.ascontiguousarray(
            np.concatenate([x[gg, r * 64:(r + 1) * 64] for gg in range(B)])
            + b2[None, :])
        for nm, W in (("wq", Wq), ("wk", Wk), ("wv", Wv), ("wf", Wf)):
            m[nm] = np.ascontiguousarray(
                W[:, jsl].reshape(DT, 128, HB, 128)
                .transpose(1, 0, 2, 3).reshape(128, 4096).astype(bf16))
        for nm, bvec in (("bqp", bq), ("bkp", bk), ("bvp", bv),
                         ("bfp", bf_)):
            m[nm] = np.ascontiguousarray(
                bvec[jsl].reshape(HB, 128).T.astype(f32))
        in_maps.append(m)
    return in_maps


def kernel(**inputs):
    if "nc" not in _cache:
        _cache["nc"], _cache["dbg"] = _build()
    nc = _cache["nc"]
    in_maps = _host_prep(inputs)
    res = run_bass_kernel_spmd(nc, in_maps, core_ids=list(range(N_CORES)),
                               trace=bool(os.environ.get("GSA_TRACE")))
    _cache["last_results"] = res
    out = np.zeros((B, T, D), np.float32)
    for r in range(N_CORES):
        yr = res.results[r]["y_out"]
        for g in range(B):
            out[g, r * 64:(r + 1) * 64, :] = yr[g * 64:(g + 1) * 64]
    return out
